# revision 34
# baseline (speedup 1.0000x reference)
"""Trainium2 Bass kernel for nn_Attention_34033320854122.

Dense transformer attention block: QKV proj -> causal depthwise conv+SiLU ->
per-head RMSNorm -> partial RoPE -> causal attention -> output projection.

Sharding: tensor-parallel over the 16 heads across 8 NeuronCores (2 heads =
256 channels per core). Each core computes q/k/v for its channels (full
contraction over D), runs attention for its 2 heads, and produces a partial
output projection (outT_partial = Wo[:, cols] @ attn_cols^T). The host sums
the 8 partials and transposes.

Fidelity notes:
- The reference negates the rotated RoPE sub-dim of BOTH q and k; the
  negation cancels in q.k and is skipped.
- softmax without max-subtraction: scores are O(1)-bounded.
- rstd = 1/sqrt(mean(x^2)) computed as exp(-0.5*ln(ms)); eps=1e-5 is
  dropped (ms is O(0.1..1), relative impact < 1e-4).
- norm weights are folded into the RoPE trig tables (rot rows) and a
  per-partition scalar (pass rows); rstd is applied post-rope (it is a
  per-position scalar, commuting with the rotation).

Scheduling: activation table-set switches are minimized (silu-set, then
natural-log/exp set for everything else). RoPE's misaligned half-rotation
products run on GpSimd; V is transposed by the DMA XBAR; per-position
reciprocal-norm rows are partition-broadcast by stride-0 DMA.
"""

from contextlib import ExitStack

import ml_dtypes
import numpy as np

import concourse.bacc as bacc
import concourse.tile as tile
import concourse.mybir as mybir
from concourse import bass_utils

# Problem shape (hardcoded per contract)
B, T, D = 1, 2048, 2048
H, HD = 16, 128
RD = 64
KCONV = 4
NCORES = 8
CPC = D // NCORES      # channels per core = 256
MPC = CPC // HD        # head tiles per core = 2
NT = 512               # free-dim tile for matmuls
NQ = T // NT           # 4 q tiles
KD = D // 128          # 16 contraction chunks
PAD = KCONV - 1        # causal conv history
HT = T // 2            # half-span for conv/silu

F32 = mybir.dt.float32
BF16 = mybir.dt.bfloat16


# Route Ln and Exp to the one activation-table set that contains both
# (natural_log_exp_and_others), so alternating Ln/Exp/softmax-Exp on the
# scalar engine does not reload tables. The pass only uses this mapping to
# pick a set per activation; walrus validates against the real act_info
# (which does contain both functions in that set).
import concourse.bacc as _bacc_mod
import concourse.hw_specs as _hw_specs

_orig_gat = _hw_specs.get_activation_tables


def _gat_lnexp(arch):
    tables = _orig_gat(arch)
    ln = mybir.ActivationFunctionType.Ln
    ex = mybir.ActivationFunctionType.Exp
    for name, s in tables.items():
        if name != "natural_log_exp_and_others":
            s.discard(ln)
            s.discard(ex)
    return tables


_hw_specs.get_activation_tables = _gat_lnexp
_bacc_mod.get_activation_tables = _gat_lnexp

_COMPILED = None
_DEBUG = False
_DEBUG_RESULTS = None


def _build():
    nc = bacc.Bacc("TRN2", target_bir_lowering=False, debug=False,
                   num_devices=NCORES)

    d = {}
    d["xT"] = nc.dram_tensor("xT", (D, T), BF16, kind="ExternalInput").ap()
    d["wqT"] = nc.dram_tensor("wqT", (D, CPC), BF16, kind="ExternalInput").ap()
    d["wkT"] = nc.dram_tensor("wkT", (D, CPC), BF16, kind="ExternalInput").ap()
    d["wvT"] = nc.dram_tensor("wvT", (D, CPC), BF16, kind="ExternalInput").ap()
    d["woT"] = nc.dram_tensor("woT", (128, MPC, D), BF16,
                              kind="ExternalInput").ap()
    # trig: [:,0]=cos*nwq, [:,1]=cos*nwk, [:,2]=swapped-sin*nwq, [:,3]=..nwk
    d["trig"] = nc.dram_tensor("trig", (64, 4, T), BF16,
                               kind="ExternalInput").ap()
    # per-head norm weights for pass rows: [:,0]=q, [:,1]=k (rows 0:64 == 1)
    d["snw"] = nc.dram_tensor("snw", (128, 2), F32,
                              kind="ExternalInput").ap()
    # conv weights packed [128, proj(3), m(2), tap(4)]
    d["convw"] = nc.dram_tensor("convw", (128, 3, MPC, KCONV), F32,
                                kind="ExternalInput").ap()
    # causal mask strip: mask[kl, j] = 1.0 iff kl <= j - 384
    d["mask4"] = nc.dram_tensor("mask4", (128, 896), BF16,
                                kind="ExternalInput").ap()
    # rope permutation lhsT: [:,0]=swap-32-halves, [:,1]=identity
    d["perm"] = nc.dram_tensor("perm", (64, 2, 64), BF16,
                               kind="ExternalInput").ap()
    outT = nc.dram_tensor("outT", (D, T), BF16,
                          kind="ExternalOutput").ap()
    dbg = {}
    if _DEBUG:
        dbg["dbg_qf"] = nc.dram_tensor(
            "dbg_qf", (128, MPC, T), BF16, kind="ExternalOutput").ap()
        dbg["dbg_kf"] = nc.dram_tensor(
            "dbg_kf", (128, MPC, T), BF16, kind="ExternalOutput").ap()
        dbg["dbg_vtr"] = nc.dram_tensor(
            "dbg_vtr", (128, MPC, NQ, 4, 128), BF16,
            kind="ExternalOutput").ap()
        dbg["dbg_svq"] = nc.dram_tensor(
            "dbg_svq", (128, MPC, T), BF16, kind="ExternalOutput").ap()
        dbg["dbg_rawq"] = nc.dram_tensor(
            "dbg_rawq", (128, MPC, T + PAD), BF16,
            kind="ExternalOutput").ap()

    inv_sqrt_hd = 1.0 / np.sqrt(HD)

    with ExitStack() as stk:
        tc = stk.enter_context(tile.TileContext(nc))
        if True:
            consts = stk.enter_context(tc.tile_pool(name="consts", bufs=1))
            rawp = stk.enter_context(tc.tile_pool(name="raw", bufs=1))
            svp = stk.enter_context(tc.tile_pool(name="sv", bufs=1))
            finp = stk.enter_context(tc.tile_pool(name="fin", bufs=1))
            wop = stk.enter_context(tc.tile_pool(name="wo", bufs=1))
            psacc = stk.enter_context(
                tc.tile_pool(name="psacc", bufs=4, space="PSUM"))
            pssum = stk.enter_context(
                tc.tile_pool(name="pssum", bufs=1, space="PSUM"))
            pssm = stk.enter_context(
                tc.tile_pool(name="pssm", bufs=3, space="PSUM"))
            # ---- constants ----
            trig_t = consts.tile([64, 4, T], BF16)
            nc.sync.dma_start(trig_t, d["trig"])
            mask4_t = consts.tile([128, 896], BF16)
            nc.scalar.dma_start(mask4_t, d["mask4"])
            convw_t = consts.tile([128, 3, MPC, KCONV], F32)
            nc.sync.dma_start(convw_t, d["convw"])
            snw_t = consts.tile([128, 2], F32)
            nc.scalar.dma_start(snw_t, d["snw"])
            ones_hd = consts.tile([128, 1], BF16)
            nc.vector.memset(ones_hd, 1.0)
            perm_t = consts.tile([64, 2, 64], BF16)
            nc.scalar.dma_start(perm_t, d["perm"])
            woT_t = wop.tile([128, MPC, D], BF16)
            nc.sync.dma_start(woT_t, d["woT"])

            # ---- persistent buffers ----
            rawq = rawp.tile([128, MPC, T + PAD], BF16)
            rawk = rawp.tile([128, MPC, T + PAD], BF16)
            rawv = rawp.tile([128, MPC, T + PAD], BF16)
            for r in (rawq, rawk, rawv):
                nc.vector.memset(r[:, :, 0:PAD], 0.0)
            raws = (rawq, rawk, rawv)
            # silu outputs (q/k get roped in place; v feeds the transpose)
            svq = svp.tile([128, MPC, T], BF16)
            svk = svp.tile([128, MPC, T], BF16)
            vv = svp.tile([128, MPC, T], BF16)
            svs = (svq, svk, vv)
            # final q/k in head-transposed layout [HD, m, T]
            qfT = finp.tile([128, MPC, T], BF16)
            kfT = finp.tile([128, MPC, T], BF16)
            fins = (qfT, kfT)
            # v^T per 512-block, stride-4 interleave: t = 512*b + 4*p + c
            vtr = finp.tile([128, MPC, NQ, 4, 128], BF16)

            groups = [(0, 0), (0, 1), (1, 0), (1, 1), (2, 0), (2, 1)]

            wqkvp = stk.enter_context(tc.tile_pool(name="wqkv", bufs=1))
            xp = stk.enter_context(tc.tile_pool(name="xb", bufs=2))
            convp = stk.enter_context(tc.tile_pool(name="conv", bufs=3))
            sqp = stk.enter_context(tc.tile_pool(name="sq", bufs=4))
            spp = stk.enter_context(tc.tile_pool(name="sp", bufs=4))
            rrp = stk.enter_context(tc.tile_pool(name="rrb", bufs=2))
            rbcp = stk.enter_context(tc.tile_pool(name="rbc", bufs=4))
            expp = stk.enter_context(tc.tile_pool(name="exp", bufs=4))
            attnp = stk.enter_context(tc.tile_pool(name="attn", bufs=2))
            ostp = stk.enter_context(tc.tile_pool(name="ostage", bufs=2))
            smp = stk.enter_context(tc.tile_pool(name="small", bufs=2))
            if True:
                w_all = wqkvp.tile([128, KD, 3, CPC], BF16)

                def phaseA_loads(tq, first=False):
                    xb = xp.tile([128, KD, NT], BF16, name="xb", tag="xb")
                    for k4 in range(0, KD, 4):
                        if first:
                            for pi, wd in enumerate((d["wqT"], d["wkT"],
                                                     d["wvT"])):
                                deng = nc.sync if (k4 + pi) % 2 == 0 \
                                    else nc.scalar
                                deng.dma_start(
                                    w_all[:, k4:k4 + 4, pi, :],
                                    wd[k4 * 128:(k4 + 4) * 128, :].rearrange(
                                        "(k p) c -> p k c", p=128))
                        deng = (nc.sync if k4 % 8 == 0 else nc.scalar) \
                            if tq < 2 else nc.sync
                        deng.dma_start(
                            xb[:, k4:k4 + 4, :],
                            d["xT"][k4 * 128:(k4 + 4) * 128,
                                    tq * NT:(tq + 1) * NT].rearrange(
                                        "(k p) t -> p k t", p=128))
                    return xb

                def phaseA_mms(tq, xb, drain_eng):
                    # 6 simultaneous accumulations (3 psacc + 3 pssm banks)
                    pst = [psacc.tile([128, NT], F32, tag="acc",
                                      name=f"accA{gi}") for gi in range(3)] \
                        + [pssm.tile([128, NT], F32, tag="sm",
                                     name=f"accB{gi}") for gi in range(3)]
                    for k in range(KD):
                        for gi, (pi, m) in enumerate(groups):
                            nc.tensor.matmul(
                                pst[gi],
                                w_all[:, k, pi, m * 128:(m + 1) * 128],
                                xb[:, k, :],
                                start=(k == 0), stop=(k == KD - 1))
                    for gi, (pi, m) in enumerate(groups):
                        dst = raws[pi][:, m,
                                       PAD + tq * NT:PAD + (tq + 1) * NT]
                        nc.vector.tensor_copy(dst, pst[gi])

                def conv_silu_sq(pi, m, h, sqtiles, quarter=None,
                                 gate=None):
                    """conv + silu (+square for q/k); half h or quarter."""
                    if quarter is None:
                        base, ln = h * HT, HT
                    else:
                        base, ln = quarter * NT, NT
                    raw = raws[pi]
                    t0 = convp.tile([128, HT], BF16, tag="cvA", name="cv0")
                    t0 = t0[:, 0:ln]
                    nc.vector.tensor_scalar_mul(
                        t0, raw[:, m, base:base + ln],
                        convw_t[:, pi, m, 0:1])
                    for j in (1, 2, 3):
                        t1 = convp.tile([128, HT], BF16,
                                        tag=("cvB", "cvA")[j % 2], name="cvj")
                        t1 = t1[:, 0:ln]
                        nc.vector.scalar_tensor_tensor(
                            t1, raw[:, m, base + j:base + j + ln],
                            convw_t[:, pi, m, j:j + 1], t0,
                            mybir.AluOpType.mult, mybir.AluOpType.add)
                        t0 = t1
                    sv = svs[pi]
                    nc.scalar.activation(
                        sv[:, m, base:base + ln], t0,
                        mybir.ActivationFunctionType.Silu,
                        bias=gate if gate is not None else 0.0)
                    if pi < 2:
                        if quarter is None:
                            sq = sqp.tile([128, HT], BF16, tag="sq")
                            sqtiles[(pi, m, h)] = sq
                            dst = sq
                        elif quarter % 2 == 0:
                            sq = sqp.tile([128, HT], BF16, tag="sq")
                            sqtiles[(pi, m, quarter // 2)] = sq
                            dst = sq[:, 0:NT]
                        else:
                            sq = sqtiles[(pi, m, quarter // 2)]
                            dst = sq[:, NT:HT]
                        nc.scalar.activation(
                            dst, sv[:, m, base:base + ln],
                            mybir.ActivationFunctionType.Square,
                            scale=inv_sqrt_hd)

                def phaseBh(h, sqtiles):
                    for m in range(MPC):
                        for pi in range(3):
                            conv_silu_sq(pi, m, h, sqtiles)

                def phaseBq(q, sqtiles, gate=None):
                    # quarter-span conv/silu/sq (t in [q*512, (q+1)*512))
                    for m in range(MPC):
                        for pi in range(3):
                            conv_silu_sq(pi, m, None, sqtiles, quarter=q,
                                         gate=gate)

                def phaseBs_pair(s0, sqtiles):
                    """Finalize slices s0, s0+1: rstd, rope -> qfT/kfT.

                    Staged so the scalar queue sees Ln x8 then Exp x8 (one
                    table load each), and GpSimd's rope products run while
                    the scalar engine computes rstd.
                    """
                    combos = [(s, m, pi) for s in (s0, s0 + 1)
                              for m in range(MPC) for pi in range(2)]
                    ps_ss, rbcs = {}, {}
                    for cb in combos:
                        s, m, pi = cb
                        sq = sqtiles[(pi, m, s // 2)]
                        ps = pssm.tile([1, NT], F32, tag="sm", name="ps_ss")
                        nc.tensor.matmul(
                            ps, ones_hd,
                            sq[:, (s % 2) * NT:(s % 2 + 1) * NT],
                            start=True, stop=True)
                        ps_ss[cb] = ps
                    for cb in combos:  # Ln batch (one table load)
                        nc.scalar.activation(
                            ps_ss[cb], ps_ss[cb],
                            mybir.ActivationFunctionType.Ln)
                    for cb in combos:  # Exp batch; rstd = exp(-0.5*ln(ms))
                        rrb = rrp.tile([1, NT], BF16, tag="rrb", name="rrb")
                        nc.scalar.activation(
                            rrb, ps_ss[cb], mybir.ActivationFunctionType.Exp,
                            scale=-0.5)
                        rbcs[cb] = rrb
                    for cb in combos:
                        # rope: sin/cos products (DVE), rotate-half via a
                        # permutation matmul accumulated with the cos part
                        s, m, pi = cb
                        rbc = rbcp.tile([128, NT], BF16, tag="rbc",
                                        name="rbc")
                        nc.gpsimd.partition_broadcast(rbc, rbcs[cb])
                        rbcs[cb] = rbc
                        sl = slice(s * NT, (s + 1) * NT)
                        sv = svs[pi][:, m, sl]
                        sp = spp.tile([64, NT], BF16, tag="sp", name="sp")
                        nc.vector.tensor_mul(sp, sv[0:64, :],
                                             trig_t[:, 2 + pi, sl])
                        cp = spp.tile([64, NT], BF16, tag="cp", name="cp")
                        nc.vector.tensor_mul(cp, sv[0:64, :],
                                             trig_t[:, pi, sl])
                        ps_rot = psacc.tile([64, NT], F32, tag="acc",
                                            name="ps_rot")
                        nc.tensor.matmul(ps_rot, perm_t[:, 0, :], sp,
                                         start=True, stop=False)
                        nc.tensor.matmul(ps_rot, perm_t[:, 1, :], cp,
                                         start=False, stop=True)
                        rbc = rbcs[cb]
                        nc.vector.scalar_tensor_tensor(
                            fins[pi][0:64, m, sl], ps_rot, 1.0,
                            rbc[0:64, :],
                            mybir.AluOpType.mult, mybir.AluOpType.mult)
                        nc.vector.scalar_tensor_tensor(
                            fins[pi][64:128, m, sl], sv[64:128, :],
                            snw_t[64:128, pi:pi + 1], rbc[64:128, :],
                            mybir.AluOpType.mult, mybir.AluOpType.mult)
                    gatez = smp.tile([128, 1], F32, tag="gate", name="gatez")
                    nc.vector.tensor_scalar_mul(
                        gatez, rbcs[combos[-1]][:, 0:1], 0.0)
                    return gatez

                def v_transpose(b):
                    for m in range(MPC):
                        nc.sync.dma_start_transpose(
                            vtr[:, m, b],
                            vv[:, m, b * NT:(b + 1) * NT])

                def phaseC(t, interleave=None):
                    qsl = slice(t * NT, (t + 1) * NT)
                    nch = 4 * (t + 1)
                    attn_m = []
                    for m in range(MPC):
                        ps_attn = psacc.tile([128, NT], F32, tag="acc",
                                             name="ps_attn")
                        ps_sum = pssum.tile([1, NT], F32, tag="sum1",
                                            name="ps_sum")

                        def qk(kc):
                            ps_s = pssm.tile([128, NT], F32, tag="sm",
                                             name="ps_s")
                            nc.tensor.matmul(
                                ps_s,
                                kfT[:, m, kc * 128:(kc + 1) * 128],
                                qfT[:, m, qsl], start=True, stop=True)
                            e = expp.tile([128, NT], BF16, tag="e", name="e")
                            nc.scalar.activation(
                                e, ps_s, mybir.ActivationFunctionType.Exp,
                                scale=inv_sqrt_hd)
                            dd = kc * 128 - t * NT
                            if dd >= 0:  # diagonal chunk: causal mask
                                nc.vector.tensor_mul(
                                    e, e, mask4_t[:, 384 - dd:896 - dd])
                            return e

                        epipe = [qk(kc) for kc in range(min(3, nch))]
                        for kc in range(nch):
                            if kc + 3 < nch:
                                epipe.append(qk(kc + 3))
                            e = epipe.pop(0)
                            b, c = kc // 4, kc % 4
                            nc.tensor.matmul(
                                ps_attn, vtr[:, m, b, c, :], e,
                                start=(kc == 0), stop=(kc == nch - 1))
                            nc.tensor.matmul(
                                ps_sum, ones_hd, e,
                                start=(kc == 0), stop=(kc == nch - 1))
                        # normalize by 1/sumexp via stride-0 DMA broadcast
                        rrf = smp.tile([1, NT], F32, tag="rrf", name="rrf")
                        nc.vector.reciprocal_approx_fast(rrf, ps_sum)
                        rrc = smp.tile([1, NT], BF16, tag="rrc", name="rrc")
                        nc.vector.tensor_copy(rrc, rrf)
                        rbc = rbcp.tile([128, NT], BF16, tag="rbc",
                                        name="rbcC")
                        nc.gpsimd.partition_broadcast(rbc, rrc)
                        am = attnp.tile([128, NT], BF16, tag="am", name="am")
                        nc.vector.tensor_mul(am, ps_attn, rbc)
                        attn_m.append(am)
                        if interleave:
                            interleave.pop(0)()
                    # output projection (wo resident)
                    for i in range(KD):
                        ps_o = psacc.tile([128, NT], F32, tag="acc",
                                          name="ps_o")
                        for j in range(MPC):
                            nc.tensor.matmul(
                                ps_o, woT_t[:, j, i * 128:(i + 1) * 128],
                                attn_m[j], start=(j == 0),
                                stop=(j == MPC - 1))
                        ost = ostp.tile([128, NT], BF16, tag="ost",
                                        name="ost")
                        if i % 4 == 3:
                            nc.scalar.activation(
                                ost, ps_o, mybir.ActivationFunctionType.Copy)
                        else:
                            nc.vector.tensor_copy(ost, ps_o)
                        deng = nc.sync if i % 2 == 0 else nc.gpsimd
                        deng.dma_start(outT[i * 128:(i + 1) * 128, qsl],
                                       ost)
                        if interleave:
                            interleave.pop(0)()

                # ================= emission schedule =================
                sqtiles = {}
                xb0 = phaseA_loads(0, first=True)
                xb1 = phaseA_loads(1)
                phaseA_mms(0, xb0, "v")
                phaseA_mms(1, xb1, "v")
                xb2 = phaseA_loads(2)
                xb3 = phaseA_loads(3)
                phaseBh(0, sqtiles)       # conv/silu/sq for t in [0, 1024)
                phaseA_mms(2, xb2, "v")
                phaseA_mms(3, xb3, "v")
                gate = phaseBs_pair(0, sqtiles)
                v_transpose(0)
                v_transpose(1)
                phaseBq(2, sqtiles, gate)  # t in [1024, 1536) (needs A2)
                phaseBq(3, sqtiles, gate)  # t in [1536, 2048)
                phaseBs_pair(2, sqtiles)
                v_transpose(2)
                v_transpose(3)
                phaseC(0)
                phaseC(1)
                phaseC(2)
                phaseC(3)
                if _DEBUG:
                    nc.sync.dma_start(dbg["dbg_qf"], qfT)
                    nc.sync.dma_start(dbg["dbg_kf"], kfT)
                    nc.sync.dma_start(dbg["dbg_vtr"], vtr)
                    nc.sync.dma_start(dbg["dbg_svq"], svq)
                    nc.sync.dma_start(dbg["dbg_rawq"], rawq)

    nc.compile()
    return nc


def _prep_inputs(hidden_states, cos, sin, Wq, Wk, Wv, Wo,
                 conv_q_w, conv_k_w, conv_v_w, q_norm_w, k_norm_w):
    f = np.float32
    bf = ml_dtypes.bfloat16
    x = np.asarray(hidden_states, f)[0]            # [T, D]
    xT = np.ascontiguousarray(x.T.astype(bf))      # [D, T] bf16
    WqT = np.ascontiguousarray(np.asarray(Wq, f).T.astype(bf))
    WkT = np.ascontiguousarray(np.asarray(Wk, f).T.astype(bf))
    WvT = np.ascontiguousarray(np.asarray(Wv, f).T.astype(bf))
    WoT = np.asarray(Wo, f).T                      # [CPC(full D), D]

    cosT = np.asarray(cos, f)[0].T                 # [RD, T]
    sinT = np.asarray(sin, f)[0].T
    nwq = np.asarray(q_norm_w, f)
    nwk = np.asarray(k_norm_w, f)

    # trig tables with norm weights folded into the rotary rows.
    # sin table indexed by SOURCE row r (out row p = r xor 32):
    #   r in 0:32  -> p = r+32: +sin[p]*nw[p]
    #   r in 32:64 -> p = r-32: -sin[p]*nw[p]
    def mk_trig(nw):
        cosb = cosT * nw[0:RD, None]
        ss = np.zeros((RD, T), f)
        ss[0:32] = sinT[32:64] * nw[32:64, None]
        ss[32:64] = -sinT[0:32] * nw[0:32, None]
        return cosb, ss

    cosq, ssq = mk_trig(nwq)
    cosk, ssk = mk_trig(nwk)
    trig = np.stack([cosq, cosk, ssq, ssk], axis=1).astype(bf)  # [64,4,T]

    snw = np.ones((128, 2), f)
    snw[RD:128, 0] = nwq[RD:128]
    snw[RD:128, 1] = nwk[RD:128]

    # causal mask strip: mask[kl, j] = 1.0 iff kl <= j - 384
    pp = np.arange(128, dtype=f)[:, None]
    jj = np.arange(896, dtype=f)[None, :]
    mask4 = (pp <= jj - 384).astype(bf)

    # rope rotate-half permutation + identity (lhsT: out = lhsT.T @ x)
    perm = np.zeros((64, 2, 64), f)
    for r in range(64):
        perm[r, 0, r ^ 32] = 1.0
        perm[r, 1, r] = 1.0
    perm = perm.astype(bf)

    in_maps = []
    for ci in range(NCORES):
        sl = slice(ci * CPC, (ci + 1) * CPC)
        convw = np.zeros((128, 3, MPC, KCONV), f)
        for pi, cw in enumerate((conv_q_w, conv_k_w, conv_v_w)):
            convw[:, pi] = np.asarray(cw, f)[sl].reshape(MPC, 128, KCONV
                                                         ).transpose(1, 0, 2)
        wo_res = np.ascontiguousarray(
            WoT[sl].reshape(MPC, 128, D).transpose(1, 0, 2).astype(bf))
        in_maps.append({
            "xT": xT,
            "wqT": np.ascontiguousarray(WqT[:, sl]),
            "wkT": np.ascontiguousarray(WkT[:, sl]),
            "wvT": np.ascontiguousarray(WvT[:, sl]),
            "woT": wo_res,
            "trig": trig,
            "snw": snw,
            "convw": np.ascontiguousarray(convw),
            "mask4": np.ascontiguousarray(mask4),
            "perm": perm,
        })
    return in_maps


def kernel(hidden_states, cos, sin, Wq, Wk, Wv, Wo,
           conv_q_w, conv_k_w, conv_v_w, q_norm_w, k_norm_w,
           _trace=False):
    global _COMPILED
    if _COMPILED is None:
        _COMPILED = _build()
    nc = _COMPILED
    in_maps = _prep_inputs(hidden_states, cos, sin, Wq, Wk, Wv, Wo,
                           conv_q_w, conv_k_w, conv_v_w, q_norm_w, k_norm_w)
    res = bass_utils.run_bass_kernel_spmd(
        nc, in_maps, core_ids=list(range(NCORES)), trace=_trace)
    if _DEBUG:
        global _DEBUG_RESULTS
        _DEBUG_RESULTS = res.results
    acc = np.zeros((D, T), np.float64)
    for r in res.results:
        acc += np.asarray(r["outT"], np.float64)
    out = np.ascontiguousarray(acc.T.astype(np.float32))[None]
    if _trace:
        kernel._last_results = res
    return out


# revision 35
# speedup vs baseline: 1.0176x; 1.0176x over previous
"""Trainium2 Bass kernel for nn_Attention_34033320854122.

Dense transformer attention block: QKV proj -> causal depthwise conv+SiLU ->
per-head RMSNorm -> partial RoPE -> causal attention -> output projection.

Sharding: tensor-parallel over the 16 heads across 8 NeuronCores (2 heads =
256 channels per core). Each core computes q/k/v for its channels (full
contraction over D), runs attention for its 2 heads, and produces a partial
output projection (outT_partial = Wo[:, cols] @ attn_cols^T). The host sums
the 8 partials and transposes.

Fidelity notes:
- The reference negates the rotated RoPE sub-dim of BOTH q and k; the
  negation cancels in q.k and is skipped.
- softmax without max-subtraction: scores are O(1)-bounded.
- rstd = 1/sqrt(mean(x^2)) computed as exp(-0.5*ln(ms)); eps=1e-5 is
  dropped (ms is O(0.1..1), relative impact < 1e-4).
- norm weights are folded into the RoPE trig tables (rot rows) and a
  per-partition scalar (pass rows); rstd is applied post-rope (it is a
  per-position scalar, commuting with the rotation).

Scheduling: activation table-set switches are minimized (silu-set, then
natural-log/exp set for everything else). RoPE's misaligned half-rotation
products run on GpSimd; V is transposed by the DMA XBAR; per-position
reciprocal-norm rows are partition-broadcast by stride-0 DMA.
"""

from contextlib import ExitStack

import ml_dtypes
import numpy as np

import concourse.bacc as bacc
import concourse.tile as tile
import concourse.mybir as mybir
from concourse import bass_utils

# Problem shape (hardcoded per contract)
B, T, D = 1, 2048, 2048
H, HD = 16, 128
RD = 64
KCONV = 4
NCORES = 8
CPC = D // NCORES      # channels per core = 256
MPC = CPC // HD        # head tiles per core = 2
NT = 512               # free-dim tile for matmuls
NQ = T // NT           # 4 q tiles
KD = D // 128          # 16 contraction chunks
PAD = KCONV - 1        # causal conv history
HT = T // 2            # half-span for conv/silu

F32 = mybir.dt.float32
BF16 = mybir.dt.bfloat16


# Route Ln and Exp to the one activation-table set that contains both
# (natural_log_exp_and_others), so alternating Ln/Exp/softmax-Exp on the
# scalar engine does not reload tables. The pass only uses this mapping to
# pick a set per activation; walrus validates against the real act_info
# (which does contain both functions in that set).
import concourse.bacc as _bacc_mod
import concourse.hw_specs as _hw_specs

_orig_gat = _hw_specs.get_activation_tables


def _gat_lnexp(arch):
    tables = _orig_gat(arch)
    ln = mybir.ActivationFunctionType.Ln
    ex = mybir.ActivationFunctionType.Exp
    for name, s in tables.items():
        if name != "natural_log_exp_and_others":
            s.discard(ln)
            s.discard(ex)
    return tables


_hw_specs.get_activation_tables = _gat_lnexp
_bacc_mod.get_activation_tables = _gat_lnexp

_COMPILED = None
_DEBUG = False
_DEBUG_RESULTS = None


def _build():
    nc = bacc.Bacc("TRN2", target_bir_lowering=False, debug=False,
                   num_devices=NCORES)

    d = {}
    d["xT"] = nc.dram_tensor("xT", (D, T), BF16, kind="ExternalInput").ap()
    d["wqT"] = nc.dram_tensor("wqT", (D, CPC), BF16, kind="ExternalInput").ap()
    d["wkT"] = nc.dram_tensor("wkT", (D, CPC), BF16, kind="ExternalInput").ap()
    d["wvT"] = nc.dram_tensor("wvT", (D, CPC), BF16, kind="ExternalInput").ap()
    d["woT"] = nc.dram_tensor("woT", (128, MPC, D), BF16,
                              kind="ExternalInput").ap()
    # trig: [:,0]=cos*nwq, [:,1]=cos*nwk, [:,2]=swapped-sin*nwq, [:,3]=..nwk
    d["trig"] = nc.dram_tensor("trig", (64, 4, T), BF16,
                               kind="ExternalInput").ap()
    # per-head norm weights for pass rows: [:,0]=q, [:,1]=k (rows 0:64 == 1)
    d["snw"] = nc.dram_tensor("snw", (128, 2), F32,
                              kind="ExternalInput").ap()
    # conv weights packed [128, proj(3), m(2), tap(4)]
    d["convw"] = nc.dram_tensor("convw", (128, 3, MPC, KCONV), F32,
                                kind="ExternalInput").ap()
    # causal mask strip: mask[kl, j] = 1.0 iff kl <= j - 384
    d["mask4"] = nc.dram_tensor("mask4", (128, 896), BF16,
                                kind="ExternalInput").ap()
    # rope permutation lhsT: [:,0]=swap-32-halves, [:,1]=identity
    d["perm"] = nc.dram_tensor("perm", (64, 2, 64), BF16,
                               kind="ExternalInput").ap()
    outT = nc.dram_tensor("outT", (D, T), BF16,
                          kind="ExternalOutput").ap()
    dbg = {}
    if _DEBUG:
        dbg["dbg_qf"] = nc.dram_tensor(
            "dbg_qf", (128, MPC, T), BF16, kind="ExternalOutput").ap()
        dbg["dbg_kf"] = nc.dram_tensor(
            "dbg_kf", (128, MPC, T), BF16, kind="ExternalOutput").ap()
        dbg["dbg_vtr"] = nc.dram_tensor(
            "dbg_vtr", (128, MPC, NQ, 4, 128), BF16,
            kind="ExternalOutput").ap()
        dbg["dbg_svq"] = nc.dram_tensor(
            "dbg_svq", (128, MPC, T), BF16, kind="ExternalOutput").ap()
        dbg["dbg_rawq"] = nc.dram_tensor(
            "dbg_rawq", (128, MPC, T + PAD), BF16,
            kind="ExternalOutput").ap()

    inv_sqrt_hd = 1.0 / np.sqrt(HD)

    with ExitStack() as stk:
        tc = stk.enter_context(tile.TileContext(nc))
        if True:
            consts = stk.enter_context(tc.tile_pool(name="consts", bufs=1))
            rawp = stk.enter_context(tc.tile_pool(name="raw", bufs=1))
            svp = stk.enter_context(tc.tile_pool(name="sv", bufs=1))
            finp = stk.enter_context(tc.tile_pool(name="fin", bufs=1))
            wop = stk.enter_context(tc.tile_pool(name="wo", bufs=1))
            psacc = stk.enter_context(
                tc.tile_pool(name="psacc", bufs=4, space="PSUM"))
            pssum = stk.enter_context(
                tc.tile_pool(name="pssum", bufs=1, space="PSUM"))
            pssm = stk.enter_context(
                tc.tile_pool(name="pssm", bufs=3, space="PSUM"))
            # ---- constants ----
            trig_t = consts.tile([64, 4, T], BF16)
            nc.sync.dma_start(trig_t, d["trig"])
            mask4_t = consts.tile([128, 896], BF16)
            nc.scalar.dma_start(mask4_t, d["mask4"])
            convw_t = consts.tile([128, 3, MPC, KCONV], F32)
            nc.sync.dma_start(convw_t, d["convw"])
            snw_t = consts.tile([128, 2], F32)
            nc.scalar.dma_start(snw_t, d["snw"])
            ones_hd = consts.tile([128, 1], BF16)
            nc.vector.memset(ones_hd, 1.0)
            perm_t = consts.tile([64, 2, 64], BF16)
            nc.scalar.dma_start(perm_t, d["perm"])
            woT_t = wop.tile([128, MPC, D], BF16)
            nc.sync.dma_start(woT_t, d["woT"])

            # ---- persistent buffers ----
            rawq = rawp.tile([128, MPC, T + PAD], BF16)
            rawk = rawp.tile([128, MPC, T + PAD], BF16)
            rawv = rawp.tile([128, MPC, T + PAD], BF16)
            for r in (rawq, rawk, rawv):
                nc.vector.memset(r[:, :, 0:PAD], 0.0)
            raws = (rawq, rawk, rawv)
            # silu outputs (q/k get roped in place; v feeds the transpose)
            svq = svp.tile([128, MPC, T], BF16)
            svk = svp.tile([128, MPC, T], BF16)
            vv = svp.tile([128, MPC, T], BF16)
            svs = (svq, svk, vv)
            # final q/k in head-transposed layout [HD, m, T]
            qfT = finp.tile([128, MPC, T], BF16)
            kfT = finp.tile([128, MPC, T], BF16)
            fins = (qfT, kfT)
            # v^T per 512-block, stride-4 interleave: t = 512*b + 4*p + c
            vtr = finp.tile([128, MPC, NQ, 4, 128], BF16)

            groups = [(0, 0), (0, 1), (1, 0), (1, 1), (2, 0), (2, 1)]

            wqkvp = stk.enter_context(tc.tile_pool(name="wqkv", bufs=1))
            xp = stk.enter_context(tc.tile_pool(name="xb", bufs=2))
            convp = stk.enter_context(tc.tile_pool(name="conv", bufs=3))
            sqp = stk.enter_context(tc.tile_pool(name="sq", bufs=4))
            spp = stk.enter_context(tc.tile_pool(name="sp", bufs=4))
            rrp = stk.enter_context(tc.tile_pool(name="rrb", bufs=2))
            rbcp = stk.enter_context(tc.tile_pool(name="rbc", bufs=4))
            expp = stk.enter_context(tc.tile_pool(name="exp", bufs=4))
            attnp = stk.enter_context(tc.tile_pool(name="attn", bufs=2))
            ostp = stk.enter_context(tc.tile_pool(name="ostage", bufs=2))
            smp = stk.enter_context(tc.tile_pool(name="small", bufs=2))
            if True:
                w_all = wqkvp.tile([128, KD, 3, CPC], BF16)

                def phaseA_loads(tq, first=False):
                    xb = xp.tile([128, KD, NT], BF16, name="xb", tag="xb")
                    for k4 in range(0, KD, 4):
                        if first:
                            for pi, wd in enumerate((d["wqT"], d["wkT"],
                                                     d["wvT"])):
                                deng = nc.sync if (k4 + pi) % 2 == 0 \
                                    else nc.scalar
                                deng.dma_start(
                                    w_all[:, k4:k4 + 4, pi, :],
                                    wd[k4 * 128:(k4 + 4) * 128, :].rearrange(
                                        "(k p) c -> p k c", p=128))
                        deng = (nc.sync if k4 % 8 == 0 else nc.scalar) \
                            if tq < 2 else nc.sync
                        deng.dma_start(
                            xb[:, k4:k4 + 4, :],
                            d["xT"][k4 * 128:(k4 + 4) * 128,
                                    tq * NT:(tq + 1) * NT].rearrange(
                                        "(k p) t -> p k t", p=128))
                    return xb

                def phaseA_mms(tq, xb, drain_eng):
                    # 6 simultaneous accumulations (3 psacc + 3 pssm banks)
                    pst = [psacc.tile([128, NT], F32, tag="acc",
                                      name=f"accA{gi}") for gi in range(3)] \
                        + [pssm.tile([128, NT], F32, tag="sm",
                                     name=f"accB{gi}") for gi in range(3)]
                    for k in range(KD):
                        for gi, (pi, m) in enumerate(groups):
                            nc.tensor.matmul(
                                pst[gi],
                                w_all[:, k, pi, m * 128:(m + 1) * 128],
                                xb[:, k, :],
                                start=(k == 0), stop=(k == KD - 1))
                    for gi, (pi, m) in enumerate(groups):
                        dst = raws[pi][:, m,
                                       PAD + tq * NT:PAD + (tq + 1) * NT]
                        nc.vector.tensor_copy(dst, pst[gi])

                def conv_silu_sq(pi, m, h, sqtiles, quarter=None,
                                 gate=None):
                    """conv + silu (+square for q/k); half h or quarter."""
                    if quarter is None:
                        base, ln = h * HT, HT
                    else:
                        base, ln = quarter * NT, NT
                    raw = raws[pi]
                    t0 = convp.tile([128, HT], BF16, tag="cvA", name="cv0")
                    t0 = t0[:, 0:ln]
                    nc.vector.tensor_scalar_mul(
                        t0, raw[:, m, base:base + ln],
                        convw_t[:, pi, m, 0:1])
                    for j in (1, 2, 3):
                        t1 = convp.tile([128, HT], BF16,
                                        tag=("cvB", "cvA")[j % 2], name="cvj")
                        t1 = t1[:, 0:ln]
                        nc.vector.scalar_tensor_tensor(
                            t1, raw[:, m, base + j:base + j + ln],
                            convw_t[:, pi, m, j:j + 1], t0,
                            mybir.AluOpType.mult, mybir.AluOpType.add)
                        t0 = t1
                    sv = svs[pi]
                    nc.scalar.activation(
                        sv[:, m, base:base + ln], t0,
                        mybir.ActivationFunctionType.Silu,
                        bias=gate if gate is not None else 0.0)
                    if pi < 2:
                        if quarter is None:
                            sq = sqp.tile([128, HT], BF16, tag="sq")
                            sqtiles[(pi, m, h)] = sq
                            dst = sq
                        elif quarter % 2 == 0:
                            sq = sqp.tile([128, HT], BF16, tag="sq")
                            sqtiles[(pi, m, quarter // 2)] = sq
                            dst = sq[:, 0:NT]
                        else:
                            sq = sqtiles[(pi, m, quarter // 2)]
                            dst = sq[:, NT:HT]
                        nc.scalar.activation(
                            dst, sv[:, m, base:base + ln],
                            mybir.ActivationFunctionType.Square,
                            scale=inv_sqrt_hd)

                def phaseBh(h, sqtiles):
                    for m in range(MPC):
                        for pi in range(3):
                            conv_silu_sq(pi, m, h, sqtiles)

                def phaseBq(q, sqtiles, gate=None):
                    # quarter-span conv/silu/sq (t in [q*512, (q+1)*512))
                    for m in range(MPC):
                        for pi in range(3):
                            conv_silu_sq(pi, m, None, sqtiles, quarter=q,
                                         gate=gate)

                def phaseBs_pair(s0, sqtiles):
                    """Finalize slices s0, s0+1: rstd, rope -> qfT/kfT.

                    Staged so the scalar queue sees Ln x8 then Exp x8 (one
                    table load each), and GpSimd's rope products run while
                    the scalar engine computes rstd.
                    """
                    combos = [(s, m, pi) for s in (s0, s0 + 1)
                              for m in range(MPC) for pi in range(2)]
                    ps_ss, rbcs = {}, {}
                    for cb in combos:
                        s, m, pi = cb
                        sq = sqtiles[(pi, m, s // 2)]
                        ps = pssm.tile([1, NT], F32, tag="sm", name="ps_ss")
                        nc.tensor.matmul(
                            ps, ones_hd,
                            sq[:, (s % 2) * NT:(s % 2 + 1) * NT],
                            start=True, stop=True)
                        ps_ss[cb] = ps
                    for cb in combos:  # Ln batch (one table load)
                        nc.scalar.activation(
                            ps_ss[cb], ps_ss[cb],
                            mybir.ActivationFunctionType.Ln)
                    for cb in combos:  # Exp batch; rstd = exp(-0.5*ln(ms))
                        rrb = rrp.tile([1, NT], BF16, tag="rrb", name="rrb")
                        nc.scalar.activation(
                            rrb, ps_ss[cb], mybir.ActivationFunctionType.Exp,
                            scale=-0.5)
                        rbcs[cb] = rrb
                    for cb in combos:
                        # rope: sin/cos products (DVE), rotate-half via a
                        # permutation matmul accumulated with the cos part
                        s, m, pi = cb
                        rbc = rbcp.tile([128, NT], BF16, tag="rbc",
                                        name="rbc")
                        nc.gpsimd.partition_broadcast(rbc, rbcs[cb])
                        rbcs[cb] = rbc
                        sl = slice(s * NT, (s + 1) * NT)
                        sv = svs[pi][:, m, sl]
                        sp = spp.tile([64, NT], BF16, tag="sp", name="sp")
                        nc.vector.tensor_mul(sp, sv[0:64, :],
                                             trig_t[:, 2 + pi, sl])
                        cp = spp.tile([64, NT], BF16, tag="cp", name="cp")
                        nc.vector.tensor_mul(cp, sv[0:64, :],
                                             trig_t[:, pi, sl])
                        ps_rot = psacc.tile([64, NT], F32, tag="acc",
                                            name="ps_rot")
                        nc.tensor.matmul(ps_rot, perm_t[:, 0, :], sp,
                                         start=True, stop=False)
                        nc.tensor.matmul(ps_rot, perm_t[:, 1, :], cp,
                                         start=False, stop=True)
                        rbc = rbcs[cb]
                        nc.vector.scalar_tensor_tensor(
                            fins[pi][0:64, m, sl], ps_rot, 1.0,
                            rbc[0:64, :],
                            mybir.AluOpType.mult, mybir.AluOpType.mult)
                        nc.vector.scalar_tensor_tensor(
                            fins[pi][64:128, m, sl], sv[64:128, :],
                            snw_t[64:128, pi:pi + 1], rbc[64:128, :],
                            mybir.AluOpType.mult, mybir.AluOpType.mult)
                    gatez = smp.tile([128, 1], F32, tag="gate", name="gatez")
                    nc.vector.tensor_scalar_mul(
                        gatez, rbcs[combos[-1]][:, 0:1], 0.0)
                    return gatez

                def v_transpose(b):
                    for m in range(MPC):
                        nc.sync.dma_start_transpose(
                            vtr[:, m, b],
                            vv[:, m, b * NT:(b + 1) * NT])

                def phaseC(t, interleave=None):
                    qsl = slice(t * NT, (t + 1) * NT)
                    nch = 4 * (t + 1)
                    attn_m = []
                    for m in range(MPC):
                        ps_attn = psacc.tile([128, NT], F32, tag="acc",
                                             name="ps_attn")
                        ps_sum = pssum.tile([1, NT], F32, tag="sum1",
                                            name="ps_sum")

                        def qk(kc):
                            ps_s = pssm.tile([128, NT], F32, tag="sm",
                                             name="ps_s")
                            nc.tensor.matmul(
                                ps_s,
                                kfT[:, m, kc * 128:(kc + 1) * 128],
                                qfT[:, m, qsl], start=True, stop=True)
                            e = expp.tile([128, NT], BF16, tag="e", name="e")
                            nc.scalar.activation(
                                e, ps_s, mybir.ActivationFunctionType.Exp,
                                scale=inv_sqrt_hd)
                            dd = kc * 128 - t * NT
                            if dd >= 0:  # diagonal chunk: causal mask
                                nc.vector.tensor_mul(
                                    e, e, mask4_t[:, 384 - dd:896 - dd])
                            return e

                        epipe = [qk(kc) for kc in range(min(3, nch))]
                        for kc in range(nch):
                            if kc + 3 < nch:
                                epipe.append(qk(kc + 3))
                            e = epipe.pop(0)
                            b, c = kc // 4, kc % 4
                            nc.tensor.matmul(
                                ps_attn, vtr[:, m, b, c, :], e,
                                start=(kc == 0), stop=(kc == nch - 1))
                            nc.tensor.matmul(
                                ps_sum, ones_hd, e,
                                start=(kc == 0), stop=(kc == nch - 1))
                        # normalize by 1/sumexp via stride-0 DMA broadcast
                        rrf = smp.tile([1, NT], F32, tag="rrf", name="rrf")
                        nc.vector.reciprocal_approx_fast(rrf, ps_sum)
                        rrc = smp.tile([1, NT], BF16, tag="rrc", name="rrc")
                        nc.vector.tensor_copy(rrc, rrf)
                        rbc = rbcp.tile([128, NT], BF16, tag="rbc",
                                        name="rbcC")
                        nc.gpsimd.partition_broadcast(rbc, rrc)
                        am = attnp.tile([128, NT], BF16, tag="am", name="am")
                        nc.vector.tensor_mul(am, ps_attn, rbc)
                        attn_m.append(am)
                        if interleave:
                            interleave.pop(0)()
                    # output projection (wo resident)
                    for i in range(KD):
                        ps_o = psacc.tile([128, NT], F32, tag="acc",
                                          name="ps_o")
                        for j in range(MPC):
                            nc.tensor.matmul(
                                ps_o, woT_t[:, j, i * 128:(i + 1) * 128],
                                attn_m[j], start=(j == 0),
                                stop=(j == MPC - 1))
                        ost = ostp.tile([128, NT], BF16, tag="ost",
                                        name="ost")
                        if i % 4 == 3:
                            nc.scalar.activation(
                                ost, ps_o, mybir.ActivationFunctionType.Copy)
                        else:
                            nc.vector.tensor_copy(ost, ps_o)
                        deng = nc.sync if i % 2 == 0 else nc.gpsimd
                        deng.dma_start(outT[i * 128:(i + 1) * 128, qsl],
                                       ost)
                        if interleave:
                            interleave.pop(0)()

                # ================= emission schedule =================
                sqtiles = {}
                xb0 = phaseA_loads(0, first=True)
                xb1 = phaseA_loads(1)
                phaseA_mms(0, xb0, "v")
                phaseA_mms(1, xb1, "v")
                xb2 = phaseA_loads(2)
                xb3 = phaseA_loads(3)
                phaseBh(0, sqtiles)       # conv/silu/sq for t in [0, 1024)
                phaseA_mms(2, xb2, "v")
                phaseBq(2, sqtiles)       # t in [1024, 1536) (needs A2)
                phaseA_mms(3, xb3, "v")
                gate = phaseBs_pair(0, sqtiles)
                phaseBq(3, sqtiles, gate)  # t in [1536, 2048)
                v_transpose(0)
                v_transpose(1)
                phaseBs_pair(2, sqtiles)
                v_transpose(2)
                v_transpose(3)
                phaseC(0)
                phaseC(1)
                phaseC(2)
                phaseC(3)
                if _DEBUG:
                    nc.sync.dma_start(dbg["dbg_qf"], qfT)
                    nc.sync.dma_start(dbg["dbg_kf"], kfT)
                    nc.sync.dma_start(dbg["dbg_vtr"], vtr)
                    nc.sync.dma_start(dbg["dbg_svq"], svq)
                    nc.sync.dma_start(dbg["dbg_rawq"], rawq)

    nc.compile()
    return nc


def _prep_inputs(hidden_states, cos, sin, Wq, Wk, Wv, Wo,
                 conv_q_w, conv_k_w, conv_v_w, q_norm_w, k_norm_w):
    f = np.float32
    bf = ml_dtypes.bfloat16
    x = np.asarray(hidden_states, f)[0]            # [T, D]
    xT = np.ascontiguousarray(x.T.astype(bf))      # [D, T] bf16
    WqT = np.ascontiguousarray(np.asarray(Wq, f).T.astype(bf))
    WkT = np.ascontiguousarray(np.asarray(Wk, f).T.astype(bf))
    WvT = np.ascontiguousarray(np.asarray(Wv, f).T.astype(bf))
    WoT = np.asarray(Wo, f).T                      # [CPC(full D), D]

    cosT = np.asarray(cos, f)[0].T                 # [RD, T]
    sinT = np.asarray(sin, f)[0].T
    nwq = np.asarray(q_norm_w, f)
    nwk = np.asarray(k_norm_w, f)

    # trig tables with norm weights folded into the rotary rows.
    # sin table indexed by SOURCE row r (out row p = r xor 32):
    #   r in 0:32  -> p = r+32: +sin[p]*nw[p]
    #   r in 32:64 -> p = r-32: -sin[p]*nw[p]
    def mk_trig(nw):
        cosb = cosT * nw[0:RD, None]
        ss = np.zeros((RD, T), f)
        ss[0:32] = sinT[32:64] * nw[32:64, None]
        ss[32:64] = -sinT[0:32] * nw[0:32, None]
        return cosb, ss

    cosq, ssq = mk_trig(nwq)
    cosk, ssk = mk_trig(nwk)
    trig = np.stack([cosq, cosk, ssq, ssk], axis=1).astype(bf)  # [64,4,T]

    snw = np.ones((128, 2), f)
    snw[RD:128, 0] = nwq[RD:128]
    snw[RD:128, 1] = nwk[RD:128]

    # causal mask strip: mask[kl, j] = 1.0 iff kl <= j - 384
    pp = np.arange(128, dtype=f)[:, None]
    jj = np.arange(896, dtype=f)[None, :]
    mask4 = (pp <= jj - 384).astype(bf)

    # rope rotate-half permutation + identity (lhsT: out = lhsT.T @ x)
    perm = np.zeros((64, 2, 64), f)
    for r in range(64):
        perm[r, 0, r ^ 32] = 1.0
        perm[r, 1, r] = 1.0
    perm = perm.astype(bf)

    in_maps = []
    for ci in range(NCORES):
        sl = slice(ci * CPC, (ci + 1) * CPC)
        convw = np.zeros((128, 3, MPC, KCONV), f)
        for pi, cw in enumerate((conv_q_w, conv_k_w, conv_v_w)):
            convw[:, pi] = np.asarray(cw, f)[sl].reshape(MPC, 128, KCONV
                                                         ).transpose(1, 0, 2)
        wo_res = np.ascontiguousarray(
            WoT[sl].reshape(MPC, 128, D).transpose(1, 0, 2).astype(bf))
        in_maps.append({
            "xT": xT,
            "wqT": np.ascontiguousarray(WqT[:, sl]),
            "wkT": np.ascontiguousarray(WkT[:, sl]),
            "wvT": np.ascontiguousarray(WvT[:, sl]),
            "woT": wo_res,
            "trig": trig,
            "snw": snw,
            "convw": np.ascontiguousarray(convw),
            "mask4": np.ascontiguousarray(mask4),
            "perm": perm,
        })
    return in_maps


def kernel(hidden_states, cos, sin, Wq, Wk, Wv, Wo,
           conv_q_w, conv_k_w, conv_v_w, q_norm_w, k_norm_w,
           _trace=False):
    global _COMPILED
    if _COMPILED is None:
        _COMPILED = _build()
    nc = _COMPILED
    in_maps = _prep_inputs(hidden_states, cos, sin, Wq, Wk, Wv, Wo,
                           conv_q_w, conv_k_w, conv_v_w, q_norm_w, k_norm_w)
    res = bass_utils.run_bass_kernel_spmd(
        nc, in_maps, core_ids=list(range(NCORES)), trace=_trace)
    if _DEBUG:
        global _DEBUG_RESULTS
        _DEBUG_RESULTS = res.results
    acc = np.zeros((D, T), np.float64)
    for r in res.results:
        acc += np.asarray(r["outT"], np.float64)
    out = np.ascontiguousarray(acc.T.astype(np.float32))[None]
    if _trace:
        kernel._last_results = res
    return out


# revision 36
# speedup vs baseline: 1.0268x; 1.0091x over previous
"""Trainium2 Bass kernel for nn_Attention_34033320854122.

Dense transformer attention block: QKV proj -> causal depthwise conv+SiLU ->
per-head RMSNorm -> partial RoPE -> causal attention -> output projection.

Sharding: tensor-parallel over the 16 heads across 8 NeuronCores (2 heads =
256 channels per core). Each core computes q/k/v for its channels (full
contraction over D), runs attention for its 2 heads, and produces a partial
output projection (outT_partial = Wo[:, cols] @ attn_cols^T). The host sums
the 8 partials and transposes.

Fidelity notes:
- The reference negates the rotated RoPE sub-dim of BOTH q and k; the
  negation cancels in q.k and is skipped.
- softmax without max-subtraction: scores are O(1)-bounded.
- rstd = 1/sqrt(mean(x^2)) computed as exp(-0.5*ln(ms)); eps=1e-5 is
  dropped (ms is O(0.1..1), relative impact < 1e-4).
- norm weights are folded into the RoPE trig tables (rot rows) and a
  per-partition scalar (pass rows); rstd is applied post-rope (it is a
  per-position scalar, commuting with the rotation).

Scheduling: activation table-set switches are minimized (silu-set first,
then one natural-log/exp set for rstd and softmax; a zero-valued gate tile
serializes the two batches). RoPE's rotate-half runs as a permutation
matmul on the PE; V is transposed by the DMA XBAR; per-position norm rows
are partition-broadcast on GpSimd.
"""

from contextlib import ExitStack

import ml_dtypes
import numpy as np

import concourse.bacc as bacc
import concourse.tile as tile
import concourse.mybir as mybir
from concourse import bass_utils

# Problem shape (hardcoded per contract)
B, T, D = 1, 2048, 2048
H, HD = 16, 128
RD = 64
KCONV = 4
NCORES = 8
CPC = D // NCORES      # channels per core = 256
MPC = CPC // HD        # head tiles per core = 2
NT = 512               # free-dim tile for matmuls
NQ = T // NT           # 4 q tiles
KD = D // 128          # 16 contraction chunks
PAD = KCONV - 1        # causal conv history
HT = T // 2            # half-span for conv/silu

F32 = mybir.dt.float32
BF16 = mybir.dt.bfloat16


# Route Ln and Exp to the one activation-table set that contains both
# (natural_log_exp_and_others), so alternating Ln/Exp/softmax-Exp on the
# scalar engine does not reload tables. The pass only uses this mapping to
# pick a set per activation; walrus validates against the real act_info
# (which does contain both functions in that set).
import concourse.bacc as _bacc_mod
import concourse.hw_specs as _hw_specs

_orig_gat = _hw_specs.get_activation_tables


def _gat_lnexp(arch):
    tables = _orig_gat(arch)
    ln = mybir.ActivationFunctionType.Ln
    ex = mybir.ActivationFunctionType.Exp
    both = tables.get("natural_log_exp_and_others")
    if both is not None and ln in both and ex in both:
        for name, s in tables.items():
            if name != "natural_log_exp_and_others":
                s.discard(ln)
                s.discard(ex)
    return tables


_hw_specs.get_activation_tables = _gat_lnexp
_bacc_mod.get_activation_tables = _gat_lnexp

_COMPILED = None
_DEBUG = False
_DEBUG_RESULTS = None


def _build():
    nc = bacc.Bacc("TRN2", target_bir_lowering=False, debug=False,
                   num_devices=NCORES)

    d = {}
    d["xT"] = nc.dram_tensor("xT", (D, T), BF16, kind="ExternalInput").ap()
    d["wqT"] = nc.dram_tensor("wqT", (D, CPC), BF16, kind="ExternalInput").ap()
    d["wkT"] = nc.dram_tensor("wkT", (D, CPC), BF16, kind="ExternalInput").ap()
    d["wvT"] = nc.dram_tensor("wvT", (D, CPC), BF16, kind="ExternalInput").ap()
    d["woT"] = nc.dram_tensor("woT", (128, MPC, D), BF16,
                              kind="ExternalInput").ap()
    # trig: [:,0]=cos*nwq, [:,1]=cos*nwk, [:,2]=swapped-sin*nwq, [:,3]=..nwk
    d["trig"] = nc.dram_tensor("trig", (64, 4, T), BF16,
                               kind="ExternalInput").ap()
    # per-head norm weights for pass rows: [:,0]=q, [:,1]=k (rows 0:64 == 1)
    d["snw"] = nc.dram_tensor("snw", (128, 2), F32,
                              kind="ExternalInput").ap()
    # conv weights packed [128, proj(3), m(2), tap(4)]
    d["convw"] = nc.dram_tensor("convw", (128, 3, MPC, KCONV), F32,
                                kind="ExternalInput").ap()
    # causal mask strip: mask[kl, j] = 1.0 iff kl <= j - 384
    d["mask4"] = nc.dram_tensor("mask4", (128, 896), BF16,
                                kind="ExternalInput").ap()
    # rope permutation lhsT: [:,0]=swap-32-halves, [:,1]=identity
    d["perm"] = nc.dram_tensor("perm", (64, 2, 64), BF16,
                               kind="ExternalInput").ap()
    outT = nc.dram_tensor("outT", (D, T), BF16,
                          kind="ExternalOutput").ap()
    dbg = {}
    if _DEBUG:
        dbg["dbg_qf"] = nc.dram_tensor(
            "dbg_qf", (128, MPC, T), BF16, kind="ExternalOutput").ap()
        dbg["dbg_kf"] = nc.dram_tensor(
            "dbg_kf", (128, MPC, T), BF16, kind="ExternalOutput").ap()
        dbg["dbg_vtr"] = nc.dram_tensor(
            "dbg_vtr", (128, MPC, NQ, 4, 128), BF16,
            kind="ExternalOutput").ap()
        dbg["dbg_svq"] = nc.dram_tensor(
            "dbg_svq", (128, MPC, T), BF16, kind="ExternalOutput").ap()
        dbg["dbg_rawq"] = nc.dram_tensor(
            "dbg_rawq", (128, MPC, T + PAD), BF16,
            kind="ExternalOutput").ap()

    inv_sqrt_hd = 1.0 / np.sqrt(HD)

    with ExitStack() as stk:
        tc = stk.enter_context(tile.TileContext(nc))
        if True:
            consts = stk.enter_context(tc.tile_pool(name="consts", bufs=1))
            rawp = stk.enter_context(tc.tile_pool(name="raw", bufs=1))
            svp = stk.enter_context(tc.tile_pool(name="sv", bufs=1))
            finp = stk.enter_context(tc.tile_pool(name="fin", bufs=1))
            wop = stk.enter_context(tc.tile_pool(name="wo", bufs=1))
            psacc = stk.enter_context(
                tc.tile_pool(name="psacc", bufs=4, space="PSUM"))
            pssum = stk.enter_context(
                tc.tile_pool(name="pssum", bufs=1, space="PSUM"))
            pssm = stk.enter_context(
                tc.tile_pool(name="pssm", bufs=3, space="PSUM"))
            # ---- constants ----
            trig_t = consts.tile([64, 4, T], BF16)
            nc.sync.dma_start(trig_t, d["trig"])
            mask4_t = consts.tile([128, 896], BF16)
            nc.scalar.dma_start(mask4_t, d["mask4"])
            convw_t = consts.tile([128, 3, MPC, KCONV], F32)
            nc.sync.dma_start(convw_t, d["convw"])
            snw_t = consts.tile([128, 2], F32)
            nc.scalar.dma_start(snw_t, d["snw"])
            ones_hd = consts.tile([128, 1], BF16)
            nc.vector.memset(ones_hd, 1.0)
            perm_t = consts.tile([64, 2, 64], BF16)
            nc.scalar.dma_start(perm_t, d["perm"])
            woT_t = wop.tile([128, MPC, D], BF16)
            nc.sync.dma_start(woT_t, d["woT"])

            # ---- persistent buffers ----
            rawq = rawp.tile([128, MPC, T + PAD], BF16)
            rawk = rawp.tile([128, MPC, T + PAD], BF16)
            rawv = rawp.tile([128, MPC, T + PAD], BF16)
            for r in (rawq, rawk, rawv):
                nc.vector.memset(r[:, :, 0:PAD], 0.0)
            raws = (rawq, rawk, rawv)
            # silu outputs (q/k get roped in place; v feeds the transpose)
            svq = svp.tile([128, MPC, T], BF16)
            svk = svp.tile([128, MPC, T], BF16)
            vv = svp.tile([128, MPC, T], BF16)
            svs = (svq, svk, vv)
            # final q/k in head-transposed layout [HD, m, T]
            qfT = finp.tile([128, MPC, T], BF16)
            kfT = finp.tile([128, MPC, T], BF16)
            fins = (qfT, kfT)
            # v^T per 512-block, stride-4 interleave: t = 512*b + 4*p + c
            vtr = finp.tile([128, MPC, NQ, 4, 128], BF16)

            groups = [(0, 0), (0, 1), (1, 0), (1, 1), (2, 0), (2, 1)]

            wqkvp = stk.enter_context(tc.tile_pool(name="wqkv", bufs=1))
            xp = stk.enter_context(tc.tile_pool(name="xb", bufs=2))
            convp = stk.enter_context(tc.tile_pool(name="conv", bufs=3))
            sqp = stk.enter_context(tc.tile_pool(name="sq", bufs=4))
            spp = stk.enter_context(tc.tile_pool(name="sp", bufs=4))
            rrp = stk.enter_context(tc.tile_pool(name="rrb", bufs=2))
            rbcp = stk.enter_context(tc.tile_pool(name="rbc", bufs=4))
            expp = stk.enter_context(tc.tile_pool(name="exp", bufs=4))
            attnp = stk.enter_context(tc.tile_pool(name="attn", bufs=2))
            ostp = stk.enter_context(tc.tile_pool(name="ostage", bufs=2))
            smp = stk.enter_context(tc.tile_pool(name="small", bufs=2))
            if True:
                w_all = wqkvp.tile([128, KD, 3, CPC], BF16)

                def phaseA_loads(tq, first=False):
                    xb = xp.tile([128, KD, NT], BF16, name="xb", tag="xb")
                    for k4 in range(0, KD, 4):
                        if first:
                            for pi, wd in enumerate((d["wqT"], d["wkT"],
                                                     d["wvT"])):
                                deng = nc.sync if (k4 + pi) % 2 == 0 \
                                    else nc.scalar
                                deng.dma_start(
                                    w_all[:, k4:k4 + 4, pi, :],
                                    wd[k4 * 128:(k4 + 4) * 128, :].rearrange(
                                        "(k p) c -> p k c", p=128))
                        deng = (nc.sync if k4 % 8 == 0 else nc.scalar) \
                            if tq < 2 else nc.sync
                        deng.dma_start(
                            xb[:, k4:k4 + 4, :],
                            d["xT"][k4 * 128:(k4 + 4) * 128,
                                    tq * NT:(tq + 1) * NT].rearrange(
                                        "(k p) t -> p k t", p=128))
                    return xb

                def phaseA_mms(tq, xb, drain_eng):
                    # 6 simultaneous accumulations (3 psacc + 3 pssm banks)
                    pst = [psacc.tile([128, NT], F32, tag="acc",
                                      name=f"accA{gi}") for gi in range(3)] \
                        + [pssm.tile([128, NT], F32, tag="sm",
                                     name=f"accB{gi}") for gi in range(3)]
                    for k in range(KD):
                        for gi, (pi, m) in enumerate(groups):
                            nc.tensor.matmul(
                                pst[gi],
                                w_all[:, k, pi, m * 128:(m + 1) * 128],
                                xb[:, k, :],
                                start=(k == 0), stop=(k == KD - 1))
                    for gi, (pi, m) in enumerate(groups):
                        dst = raws[pi][:, m,
                                       PAD + tq * NT:PAD + (tq + 1) * NT]
                        nc.vector.tensor_copy(dst, pst[gi])

                def conv_silu_sq(pi, m, h, sqtiles, quarter=None,
                                 gate=None):
                    """conv + silu (+square for q/k); half h or quarter."""
                    if quarter is None:
                        base, ln = h * HT, HT
                    else:
                        base, ln = quarter * NT, NT
                    raw = raws[pi]
                    t0 = convp.tile([128, HT], BF16, tag="cvA", name="cv0")
                    t0 = t0[:, 0:ln]
                    nc.vector.tensor_scalar_mul(
                        t0, raw[:, m, base:base + ln],
                        convw_t[:, pi, m, 0:1])
                    for j in (1, 2, 3):
                        t1 = convp.tile([128, HT], BF16,
                                        tag=("cvB", "cvA")[j % 2], name="cvj")
                        t1 = t1[:, 0:ln]
                        nc.vector.scalar_tensor_tensor(
                            t1, raw[:, m, base + j:base + j + ln],
                            convw_t[:, pi, m, j:j + 1], t0,
                            mybir.AluOpType.mult, mybir.AluOpType.add)
                        t0 = t1
                    sv = svs[pi]
                    nc.scalar.activation(
                        sv[:, m, base:base + ln], t0,
                        mybir.ActivationFunctionType.Silu,
                        bias=gate if gate is not None else 0.0)
                    if pi < 2:
                        if quarter is None:
                            sq = sqp.tile([128, HT], BF16, tag="sq")
                            sqtiles[(pi, m, h)] = sq
                            dst = sq
                        elif quarter % 2 == 0:
                            sq = sqp.tile([128, HT], BF16, tag="sq")
                            sqtiles[(pi, m, quarter // 2)] = sq
                            dst = sq[:, 0:NT]
                        else:
                            sq = sqtiles[(pi, m, quarter // 2)]
                            dst = sq[:, NT:HT]
                        nc.scalar.activation(
                            dst, sv[:, m, base:base + ln],
                            mybir.ActivationFunctionType.Square,
                            scale=inv_sqrt_hd)

                def phaseBh(h, sqtiles):
                    for m in range(MPC):
                        for pi in range(3):
                            conv_silu_sq(pi, m, h, sqtiles)

                def phaseBq(q, sqtiles, gate=None):
                    # quarter-span conv/silu/sq (t in [q*512, (q+1)*512))
                    for m in range(MPC):
                        for pi in range(3):
                            conv_silu_sq(pi, m, None, sqtiles, quarter=q,
                                         gate=gate)

                def phaseBs_pair(s0, sqtiles):
                    """Finalize slices s0, s0+1: rstd, rope -> qfT/kfT.

                    Staged so the scalar queue sees Ln x8 then Exp x8 (one
                    table load each), and GpSimd's rope products run while
                    the scalar engine computes rstd.
                    """
                    combos = [(s, m, pi) for s in (s0, s0 + 1)
                              for m in range(MPC) for pi in range(2)]
                    ps_ss, rbcs = {}, {}
                    for cb in combos:
                        s, m, pi = cb
                        sq = sqtiles[(pi, m, s // 2)]
                        ps = pssm.tile([1, NT], F32, tag="sm", name="ps_ss")
                        nc.tensor.matmul(
                            ps, ones_hd,
                            sq[:, (s % 2) * NT:(s % 2 + 1) * NT],
                            start=True, stop=True)
                        ps_ss[cb] = ps
                    for cb in combos:  # Ln batch (one table load)
                        nc.scalar.activation(
                            ps_ss[cb], ps_ss[cb],
                            mybir.ActivationFunctionType.Ln)
                    for cb in combos:  # Exp batch; rstd = exp(-0.5*ln(ms))
                        rrb = rrp.tile([1, NT], BF16, tag="rrb", name="rrb")
                        nc.scalar.activation(
                            rrb, ps_ss[cb], mybir.ActivationFunctionType.Exp,
                            scale=-0.5)
                        rbcs[cb] = rrb
                    for cb in combos:
                        # rope: sin/cos products (DVE), rotate-half via a
                        # permutation matmul accumulated with the cos part
                        s, m, pi = cb
                        rbc = rbcp.tile([128, NT], BF16, tag="rbc",
                                        name="rbc")
                        nc.gpsimd.partition_broadcast(rbc, rbcs[cb])
                        rbcs[cb] = rbc
                        sl = slice(s * NT, (s + 1) * NT)
                        sv = svs[pi][:, m, sl]
                        sp = spp.tile([64, NT], BF16, tag="sp", name="sp")
                        nc.vector.tensor_mul(sp, sv[0:64, :],
                                             trig_t[:, 2 + pi, sl])
                        cp = spp.tile([64, NT], BF16, tag="cp", name="cp")
                        nc.vector.tensor_mul(cp, sv[0:64, :],
                                             trig_t[:, pi, sl])
                        ps_rot = psacc.tile([64, NT], F32, tag="acc",
                                            name="ps_rot")
                        nc.tensor.matmul(ps_rot, perm_t[:, 0, :], sp,
                                         start=True, stop=False)
                        nc.tensor.matmul(ps_rot, perm_t[:, 1, :], cp,
                                         start=False, stop=True)
                        rbc = rbcs[cb]
                        nc.vector.scalar_tensor_tensor(
                            fins[pi][0:64, m, sl], ps_rot, 1.0,
                            rbc[0:64, :],
                            mybir.AluOpType.mult, mybir.AluOpType.mult)
                        nc.vector.scalar_tensor_tensor(
                            fins[pi][64:128, m, sl], sv[64:128, :],
                            snw_t[64:128, pi:pi + 1], rbc[64:128, :],
                            mybir.AluOpType.mult, mybir.AluOpType.mult)
                    gatez = smp.tile([128, 1], F32, tag="gate", name="gatez")
                    nc.vector.tensor_scalar_mul(
                        gatez, rbcs[combos[-1]][:, 0:1], 0.0)
                    return gatez

                def v_transpose(b):
                    for m in range(MPC):
                        nc.sync.dma_start_transpose(
                            vtr[:, m, b],
                            vv[:, m, b * NT:(b + 1) * NT])

                def phaseC(t, interleave=None):
                    qsl = slice(t * NT, (t + 1) * NT)
                    nch = 4 * (t + 1)
                    attn_m = []
                    for m in range(MPC):
                        ps_attn = psacc.tile([128, NT], F32, tag="acc",
                                             name="ps_attn")
                        ps_sum = pssum.tile([1, NT], F32, tag="sum1",
                                            name="ps_sum")

                        def qk(kc):
                            ps_s = pssm.tile([128, NT], F32, tag="sm",
                                             name="ps_s")
                            nc.tensor.matmul(
                                ps_s,
                                kfT[:, m, kc * 128:(kc + 1) * 128],
                                qfT[:, m, qsl], start=True, stop=True)
                            e = expp.tile([128, NT], BF16, tag="e", name="e")
                            nc.scalar.activation(
                                e, ps_s, mybir.ActivationFunctionType.Exp,
                                scale=inv_sqrt_hd)
                            dd = kc * 128 - t * NT
                            if dd >= 0:  # diagonal chunk: causal mask
                                nc.vector.tensor_mul(
                                    e, e, mask4_t[:, 384 - dd:896 - dd])
                            return e

                        epipe = [qk(kc) for kc in range(min(3, nch))]
                        for kc in range(nch):
                            if kc + 3 < nch:
                                epipe.append(qk(kc + 3))
                            e = epipe.pop(0)
                            b, c = kc // 4, kc % 4
                            nc.tensor.matmul(
                                ps_attn, vtr[:, m, b, c, :], e,
                                start=(kc == 0), stop=(kc == nch - 1))
                            nc.tensor.matmul(
                                ps_sum, ones_hd, e,
                                start=(kc == 0), stop=(kc == nch - 1))
                        # normalize by 1/sumexp via stride-0 DMA broadcast
                        rrf = smp.tile([1, NT], F32, tag="rrf", name="rrf")
                        nc.vector.reciprocal_approx_fast(rrf, ps_sum)
                        rrc = smp.tile([1, NT], BF16, tag="rrc", name="rrc")
                        nc.vector.tensor_copy(rrc, rrf)
                        rbc = rbcp.tile([128, NT], BF16, tag="rbc",
                                        name="rbcC")
                        nc.gpsimd.partition_broadcast(rbc, rrc)
                        am = attnp.tile([128, NT], BF16, tag="am", name="am")
                        nc.vector.tensor_mul(am, ps_attn, rbc)
                        attn_m.append(am)
                        if interleave:
                            interleave.pop(0)()
                    # output projection (wo resident)
                    for i in range(KD):
                        ps_o = psacc.tile([128, NT], F32, tag="acc",
                                          name="ps_o")
                        for j in range(MPC):
                            nc.tensor.matmul(
                                ps_o, woT_t[:, j, i * 128:(i + 1) * 128],
                                attn_m[j], start=(j == 0),
                                stop=(j == MPC - 1))
                        ost = ostp.tile([128, NT], BF16, tag="ost",
                                        name="ost")
                        if i % 4 == 3:
                            nc.scalar.activation(
                                ost, ps_o, mybir.ActivationFunctionType.Copy)
                        else:
                            nc.vector.tensor_copy(ost, ps_o)
                        deng = nc.sync if i % 2 == 0 else nc.gpsimd
                        deng.dma_start(outT[i * 128:(i + 1) * 128, qsl],
                                       ost)
                        if interleave:
                            interleave.pop(0)()

                # ================= emission schedule =================
                sqtiles = {}
                xb0 = phaseA_loads(0, first=True)
                xb1 = phaseA_loads(1)
                phaseA_mms(0, xb0, "v")
                phaseA_mms(1, xb1, "v")
                xb2 = phaseA_loads(2)
                xb3 = phaseA_loads(3)
                phaseBh(0, sqtiles)       # conv/silu/sq for t in [0, 1024)
                phaseA_mms(2, xb2, "v")
                phaseBq(2, sqtiles)       # t in [1024, 1536) (needs A2)
                phaseA_mms(3, xb3, "v")
                gate = phaseBs_pair(0, sqtiles)
                phaseBq(3, sqtiles, gate)  # t in [1536, 2048)
                v_transpose(0)
                v_transpose(1)
                phaseBs_pair(2, sqtiles)
                v_transpose(2)
                v_transpose(3)
                phaseC(0)
                phaseC(1)
                phaseC(2)
                phaseC(3)
                if _DEBUG:
                    nc.sync.dma_start(dbg["dbg_qf"], qfT)
                    nc.sync.dma_start(dbg["dbg_kf"], kfT)
                    nc.sync.dma_start(dbg["dbg_vtr"], vtr)
                    nc.sync.dma_start(dbg["dbg_svq"], svq)
                    nc.sync.dma_start(dbg["dbg_rawq"], rawq)

    nc.compile()
    return nc


def _prep_inputs(hidden_states, cos, sin, Wq, Wk, Wv, Wo,
                 conv_q_w, conv_k_w, conv_v_w, q_norm_w, k_norm_w):
    f = np.float32
    bf = ml_dtypes.bfloat16
    x = np.asarray(hidden_states, f)[0]            # [T, D]
    xT = np.ascontiguousarray(x.T.astype(bf))      # [D, T] bf16
    WqT = np.ascontiguousarray(np.asarray(Wq, f).T.astype(bf))
    WkT = np.ascontiguousarray(np.asarray(Wk, f).T.astype(bf))
    WvT = np.ascontiguousarray(np.asarray(Wv, f).T.astype(bf))
    WoT = np.asarray(Wo, f).T                      # [CPC(full D), D]

    cosT = np.asarray(cos, f)[0].T                 # [RD, T]
    sinT = np.asarray(sin, f)[0].T
    nwq = np.asarray(q_norm_w, f)
    nwk = np.asarray(k_norm_w, f)

    # trig tables with norm weights folded into the rotary rows.
    # sin table indexed by SOURCE row r (out row p = r xor 32):
    #   r in 0:32  -> p = r+32: +sin[p]*nw[p]
    #   r in 32:64 -> p = r-32: -sin[p]*nw[p]
    def mk_trig(nw):
        cosb = cosT * nw[0:RD, None]
        ss = np.zeros((RD, T), f)
        ss[0:32] = sinT[32:64] * nw[32:64, None]
        ss[32:64] = -sinT[0:32] * nw[0:32, None]
        return cosb, ss

    cosq, ssq = mk_trig(nwq)
    cosk, ssk = mk_trig(nwk)
    trig = np.stack([cosq, cosk, ssq, ssk], axis=1).astype(bf)  # [64,4,T]

    snw = np.ones((128, 2), f)
    snw[RD:128, 0] = nwq[RD:128]
    snw[RD:128, 1] = nwk[RD:128]

    # causal mask strip: mask[kl, j] = 1.0 iff kl <= j - 384
    pp = np.arange(128, dtype=f)[:, None]
    jj = np.arange(896, dtype=f)[None, :]
    mask4 = (pp <= jj - 384).astype(bf)

    # rope rotate-half permutation + identity (lhsT: out = lhsT.T @ x)
    perm = np.zeros((64, 2, 64), f)
    for r in range(64):
        perm[r, 0, r ^ 32] = 1.0
        perm[r, 1, r] = 1.0
    perm = perm.astype(bf)

    in_maps = []
    for ci in range(NCORES):
        sl = slice(ci * CPC, (ci + 1) * CPC)
        convw = np.zeros((128, 3, MPC, KCONV), f)
        for pi, cw in enumerate((conv_q_w, conv_k_w, conv_v_w)):
            convw[:, pi] = np.asarray(cw, f)[sl].reshape(MPC, 128, KCONV
                                                         ).transpose(1, 0, 2)
        wo_res = np.ascontiguousarray(
            WoT[sl].reshape(MPC, 128, D).transpose(1, 0, 2).astype(bf))
        in_maps.append({
            "xT": xT,
            "wqT": np.ascontiguousarray(WqT[:, sl]),
            "wkT": np.ascontiguousarray(WkT[:, sl]),
            "wvT": np.ascontiguousarray(WvT[:, sl]),
            "woT": wo_res,
            "trig": trig,
            "snw": snw,
            "convw": np.ascontiguousarray(convw),
            "mask4": np.ascontiguousarray(mask4),
            "perm": perm,
        })
    return in_maps


def kernel(hidden_states, cos, sin, Wq, Wk, Wv, Wo,
           conv_q_w, conv_k_w, conv_v_w, q_norm_w, k_norm_w,
           _trace=False):
    global _COMPILED
    if _COMPILED is None:
        _COMPILED = _build()
    nc = _COMPILED
    in_maps = _prep_inputs(hidden_states, cos, sin, Wq, Wk, Wv, Wo,
                           conv_q_w, conv_k_w, conv_v_w, q_norm_w, k_norm_w)
    res = bass_utils.run_bass_kernel_spmd(
        nc, in_maps, core_ids=list(range(NCORES)), trace=_trace)
    if _DEBUG:
        global _DEBUG_RESULTS
        _DEBUG_RESULTS = res.results
    acc = np.zeros((D, T), np.float64)
    for r in res.results:
        acc += np.asarray(r["outT"], np.float64)
    out = np.ascontiguousarray(acc.T.astype(np.float32))[None]
    if _trace:
        kernel._last_results = res
    return out


# revision 37
# speedup vs baseline: 1.1376x; 1.1079x over previous
"""Trainium2 Bass kernel for nn_Attention_34033320854122.

Dense transformer attention block: QKV proj -> causal depthwise conv+SiLU ->
per-head RMSNorm -> partial RoPE -> causal attention -> output projection.

Sharding: tensor-parallel over the 16 heads across 8 NeuronCores (2 heads =
256 channels per core). Each core computes q/k/v for its channels (full
contraction over D), runs attention for its 2 heads, and produces a partial
output projection (outT_partial = Wo[:, cols] @ attn_cols^T). The host sums
the 8 partials and transposes.

Fidelity notes:
- The reference negates the rotated RoPE sub-dim of BOTH q and k; the
  negation cancels in q.k and is skipped.
- softmax without max-subtraction: scores are O(1)-bounded.
- rstd = 1/sqrt(mean(x^2)) computed as exp(-0.5*ln(ms)); eps=1e-5 is
  dropped (ms is O(0.1..1), relative impact < 1e-4).
- norm weights are folded into the RoPE trig tables (rot rows) and a
  per-partition scalar (pass rows); rstd is applied post-rope (it is a
  per-position scalar, commuting with the rotation).

Scheduling: activation table-set switches are minimized (silu-set first,
then one natural-log/exp set for rstd and softmax; a zero-valued gate tile
serializes the two batches). RoPE's rotate-half runs as a permutation
matmul on the PE; V is transposed by the DMA XBAR; per-position norm rows
are partition-broadcast on GpSimd.
"""

from contextlib import ExitStack

import ml_dtypes
import numpy as np

import concourse.bacc as bacc
import concourse.tile as tile
import concourse.mybir as mybir
from concourse import bass_utils

# Problem shape (hardcoded per contract)
B, T, D = 1, 2048, 2048
H, HD = 16, 128
RD = 64
KCONV = 4
NCORES = 8
CPC = D // NCORES      # channels per core = 256
MPC = CPC // HD        # head tiles per core = 2
NT = 512               # free-dim tile for matmuls
NQ = T // NT           # 4 q tiles
KD = D // 128          # 16 contraction chunks
PAD = KCONV - 1        # causal conv history
HT = T // 2            # half-span for conv/silu

F32 = mybir.dt.float32
BF16 = mybir.dt.bfloat16


# Route Ln and Exp to the one activation-table set that contains both
# (natural_log_exp_and_others), so alternating Ln/Exp/softmax-Exp on the
# scalar engine does not reload tables. The pass only uses this mapping to
# pick a set per activation; walrus validates against the real act_info
# (which does contain both functions in that set).
import concourse.bacc as _bacc_mod
import concourse.hw_specs as _hw_specs

_orig_gat = _hw_specs.get_activation_tables


def _gat_lnexp(arch):
    tables = _orig_gat(arch)
    ln = mybir.ActivationFunctionType.Ln
    ex = mybir.ActivationFunctionType.Exp
    both = tables.get("natural_log_exp_and_others")
    if both is not None and ln in both and ex in both:
        for name, s in tables.items():
            if name != "natural_log_exp_and_others":
                s.discard(ln)
                s.discard(ex)
    return tables


_hw_specs.get_activation_tables = _gat_lnexp
_bacc_mod.get_activation_tables = _gat_lnexp

_COMPILED = None
_DEBUG = False
_DEBUG_RESULTS = None


def _build():
    nc = bacc.Bacc("TRN2", target_bir_lowering=False, debug=False,
                   num_devices=NCORES)

    d = {}
    d["xT"] = nc.dram_tensor("xT", (D, T), BF16, kind="ExternalInput").ap()
    d["wqT"] = nc.dram_tensor("wqT", (D, CPC), BF16, kind="ExternalInput").ap()
    d["wkT"] = nc.dram_tensor("wkT", (D, CPC), BF16, kind="ExternalInput").ap()
    d["wvT"] = nc.dram_tensor("wvT", (D, CPC), BF16, kind="ExternalInput").ap()
    d["woT"] = nc.dram_tensor("woT", (128, MPC, D), BF16,
                              kind="ExternalInput").ap()
    # trig: [:,0]=cos*nwq, [:,1]=cos*nwk, [:,2]=swapped-sin*nwq, [:,3]=..nwk
    d["trig"] = nc.dram_tensor("trig", (64, 4, T), BF16,
                               kind="ExternalInput").ap()
    # per-head norm weights for pass rows: [:,0]=q, [:,1]=k (rows 0:64 == 1)
    d["snw"] = nc.dram_tensor("snw", (128, 2), F32,
                              kind="ExternalInput").ap()
    # conv weights packed [128, proj(3), m(2), tap(4)]
    d["convw"] = nc.dram_tensor("convw", (128, 3, MPC, KCONV), F32,
                                kind="ExternalInput").ap()
    # causal mask strip: mask[kl, j] = 1.0 iff kl <= j - 384
    d["mask4"] = nc.dram_tensor("mask4", (128, 896), BF16,
                                kind="ExternalInput").ap()
    # rope permutation lhsT: [:,0]=swap-32-halves, [:,1]=identity
    d["perm"] = nc.dram_tensor("perm", (64, 2, 64), BF16,
                               kind="ExternalInput").ap()
    outT = nc.dram_tensor("outT", (D, T), BF16,
                          kind="ExternalOutput").ap()
    dbg = {}
    if _DEBUG:
        dbg["dbg_qf"] = nc.dram_tensor(
            "dbg_qf", (128, MPC, T), BF16, kind="ExternalOutput").ap()
        dbg["dbg_kf"] = nc.dram_tensor(
            "dbg_kf", (128, MPC, T), BF16, kind="ExternalOutput").ap()
        dbg["dbg_vtr"] = nc.dram_tensor(
            "dbg_vtr", (128, MPC, NQ, 4, 128), BF16,
            kind="ExternalOutput").ap()
        dbg["dbg_svq"] = nc.dram_tensor(
            "dbg_svq", (128, MPC, T), BF16, kind="ExternalOutput").ap()
        dbg["dbg_rawq"] = nc.dram_tensor(
            "dbg_rawq", (128, MPC, T + PAD), BF16,
            kind="ExternalOutput").ap()

    inv_sqrt_hd = 1.0 / np.sqrt(HD)

    with ExitStack() as stk:
        tc = stk.enter_context(tile.TileContext(nc))
        if True:
            consts = stk.enter_context(tc.tile_pool(name="consts", bufs=1))
            rawp = stk.enter_context(tc.tile_pool(name="raw", bufs=1))
            svp = stk.enter_context(tc.tile_pool(name="sv", bufs=1))
            finp = stk.enter_context(tc.tile_pool(name="fin", bufs=1))
            wop = stk.enter_context(tc.tile_pool(name="wo", bufs=1))
            psacc = stk.enter_context(
                tc.tile_pool(name="psacc", bufs=2, space="PSUM"))
            psout = stk.enter_context(
                tc.tile_pool(name="psout", bufs=2, space="PSUM"))
            pssum = stk.enter_context(
                tc.tile_pool(name="pssum", bufs=1, space="PSUM"))
            pssm = stk.enter_context(
                tc.tile_pool(name="pssm", bufs=3, space="PSUM"))
            # ---- constants ----
            trig_t = consts.tile([64, 4, T], BF16)
            nc.sync.dma_start(trig_t, d["trig"])
            mask4_t = consts.tile([128, 896], BF16)
            nc.scalar.dma_start(mask4_t, d["mask4"])
            convw_t = consts.tile([128, 3, MPC, KCONV], F32)
            nc.sync.dma_start(convw_t, d["convw"])
            snw_t = consts.tile([128, 2], F32)
            nc.scalar.dma_start(snw_t, d["snw"])
            ones_hd = consts.tile([128, 1], BF16)
            nc.vector.memset(ones_hd, 1.0)
            perm_t = consts.tile([64, 2, 64], BF16)
            nc.scalar.dma_start(perm_t, d["perm"])
            woT_t = wop.tile([128, MPC, D], BF16)
            nc.sync.dma_start(woT_t, d["woT"])

            # ---- persistent buffers ----
            rawq = rawp.tile([128, MPC, T + PAD], BF16)
            rawk = rawp.tile([128, MPC, T + PAD], BF16)
            rawv = rawp.tile([128, MPC, T + PAD], BF16)
            for r in (rawq, rawk, rawv):
                nc.vector.memset(r[:, :, 0:PAD], 0.0)
            raws = (rawq, rawk, rawv)
            # silu outputs (q/k get roped in place; v feeds the transpose)
            svq = svp.tile([128, MPC, T], BF16)
            svk = svp.tile([128, MPC, T], BF16)
            vv = svp.tile([128, MPC, T], BF16)
            svs = (svq, svk, vv)
            # final q/k in head-transposed layout [HD, m, T]
            qfT = finp.tile([128, MPC, T], BF16)
            kfT = finp.tile([128, MPC, T], BF16)
            fins = (qfT, kfT)
            # v^T per 512-block, stride-4 interleave: t = 512*b + 4*p + c
            vtr = finp.tile([128, MPC, NQ, 4, 128], BF16)

            groups = [(0, 0), (0, 1), (1, 0), (1, 1), (2, 0), (2, 1)]

            wqkvp = stk.enter_context(tc.tile_pool(name="wqkv", bufs=1))
            xp = stk.enter_context(tc.tile_pool(name="xb", bufs=2))
            convp = stk.enter_context(tc.tile_pool(name="conv", bufs=3))
            sqp = stk.enter_context(tc.tile_pool(name="sq", bufs=4))
            spp = stk.enter_context(tc.tile_pool(name="sp", bufs=4))
            rrp = stk.enter_context(tc.tile_pool(name="rrb", bufs=2))
            rbcp = stk.enter_context(tc.tile_pool(name="rbc", bufs=4))
            expp = stk.enter_context(tc.tile_pool(name="exp", bufs=4))
            attnp = stk.enter_context(tc.tile_pool(name="attn", bufs=2))
            ostp = stk.enter_context(tc.tile_pool(name="ostage", bufs=2))
            smp = stk.enter_context(tc.tile_pool(name="small", bufs=2))
            if True:
                w_all = wqkvp.tile([128, KD, 3, CPC], BF16)

                def phaseA_loads(tq, first=False):
                    xb = xp.tile([128, KD, NT], BF16, name="xb", tag="xb")
                    for k4 in range(0, KD, 4):
                        if first:
                            for pi, wd in enumerate((d["wqT"], d["wkT"],
                                                     d["wvT"])):
                                deng = nc.sync if (k4 + pi) % 2 == 0 \
                                    else nc.scalar
                                deng.dma_start(
                                    w_all[:, k4:k4 + 4, pi, :],
                                    wd[k4 * 128:(k4 + 4) * 128, :].rearrange(
                                        "(k p) c -> p k c", p=128))
                        deng = (nc.sync if k4 % 8 == 0 else nc.scalar) \
                            if tq < 2 else nc.sync
                        deng.dma_start(
                            xb[:, k4:k4 + 4, :],
                            d["xT"][k4 * 128:(k4 + 4) * 128,
                                    tq * NT:(tq + 1) * NT].rearrange(
                                        "(k p) t -> p k t", p=128))
                    return xb

                def phaseA_mms(tq, xb, drain_eng):
                    # 6 simultaneous accumulations (3 psacc + 3 pssm banks)
                    pst = [psacc.tile([128, NT], F32, tag="acc",
                                      name=f"accA{gi}") for gi in range(2)] \
                        + [psout.tile([128, NT], F32, tag="out",
                                      name=f"accO{gi}") for gi in range(1)] \
                        + [pssm.tile([128, NT], F32, tag="sm",
                                     name=f"accB{gi}") for gi in range(3)]
                    for k in range(KD):
                        for gi, (pi, m) in enumerate(groups):
                            nc.tensor.matmul(
                                pst[gi],
                                w_all[:, k, pi, m * 128:(m + 1) * 128],
                                xb[:, k, :],
                                start=(k == 0), stop=(k == KD - 1))
                    for gi, (pi, m) in enumerate(groups):
                        dst = raws[pi][:, m,
                                       PAD + tq * NT:PAD + (tq + 1) * NT]
                        nc.vector.tensor_copy(dst, pst[gi])

                def conv_silu_sq(pi, m, h, sqtiles, quarter=None,
                                 gate=None):
                    """conv + silu (+square for q/k); half h or quarter."""
                    if quarter is None:
                        base, ln = h * HT, HT
                    else:
                        base, ln = quarter * NT, NT
                    raw = raws[pi]
                    t0 = convp.tile([128, HT], BF16, tag="cvA", name="cv0")
                    t0 = t0[:, 0:ln]
                    nc.vector.tensor_scalar_mul(
                        t0, raw[:, m, base:base + ln],
                        convw_t[:, pi, m, 0:1])
                    for j in (1, 2, 3):
                        t1 = convp.tile([128, HT], BF16,
                                        tag=("cvB", "cvA")[j % 2], name="cvj")
                        t1 = t1[:, 0:ln]
                        nc.vector.scalar_tensor_tensor(
                            t1, raw[:, m, base + j:base + j + ln],
                            convw_t[:, pi, m, j:j + 1], t0,
                            mybir.AluOpType.mult, mybir.AluOpType.add)
                        t0 = t1
                    sv = svs[pi]
                    nc.scalar.activation(
                        sv[:, m, base:base + ln], t0,
                        mybir.ActivationFunctionType.Silu,
                        bias=gate if gate is not None else 0.0)
                    if pi < 2:
                        if quarter is None:
                            sq = sqp.tile([128, HT], BF16, tag="sq")
                            sqtiles[(pi, m, h)] = sq
                            dst = sq
                        elif quarter % 2 == 0:
                            sq = sqp.tile([128, HT], BF16, tag="sq")
                            sqtiles[(pi, m, quarter // 2)] = sq
                            dst = sq[:, 0:NT]
                        else:
                            sq = sqtiles[(pi, m, quarter // 2)]
                            dst = sq[:, NT:HT]
                        nc.scalar.activation(
                            dst, sv[:, m, base:base + ln],
                            mybir.ActivationFunctionType.Square,
                            scale=inv_sqrt_hd)

                def phaseBh(h, sqtiles):
                    for m in range(MPC):
                        for pi in range(3):
                            conv_silu_sq(pi, m, h, sqtiles)

                def phaseBq(q, sqtiles, gate=None):
                    # quarter-span conv/silu/sq (t in [q*512, (q+1)*512))
                    for m in range(MPC):
                        for pi in range(3):
                            conv_silu_sq(pi, m, None, sqtiles, quarter=q,
                                         gate=gate)

                def phaseBs_pair(s0, sqtiles):
                    """Finalize slices s0, s0+1: rstd, rope -> qfT/kfT.

                    Staged so the scalar queue sees Ln x8 then Exp x8 (one
                    table load each), and GpSimd's rope products run while
                    the scalar engine computes rstd.
                    """
                    combos = [(s, m, pi) for s in (s0, s0 + 1)
                              for m in range(MPC) for pi in range(2)]
                    ps_ss, rbcs = {}, {}
                    for cb in combos:
                        s, m, pi = cb
                        sq = sqtiles[(pi, m, s // 2)]
                        ps = pssm.tile([1, NT], F32, tag="sm", name="ps_ss")
                        nc.tensor.matmul(
                            ps, ones_hd,
                            sq[:, (s % 2) * NT:(s % 2 + 1) * NT],
                            start=True, stop=True)
                        ps_ss[cb] = ps
                    for cb in combos:  # Ln batch (one table load)
                        nc.scalar.activation(
                            ps_ss[cb], ps_ss[cb],
                            mybir.ActivationFunctionType.Ln)
                    for cb in combos:  # Exp batch; rstd = exp(-0.5*ln(ms))
                        rrb = rrp.tile([1, NT], BF16, tag="rrb", name="rrb")
                        nc.scalar.activation(
                            rrb, ps_ss[cb], mybir.ActivationFunctionType.Exp,
                            scale=-0.5)
                        rbcs[cb] = rrb
                    for cb in combos:
                        # rope: sin/cos products (DVE), rotate-half via a
                        # permutation matmul accumulated with the cos part
                        s, m, pi = cb
                        rbc = rbcp.tile([128, NT], BF16, tag="rbc",
                                        name="rbc")
                        nc.gpsimd.partition_broadcast(rbc, rbcs[cb])
                        rbcs[cb] = rbc
                        sl = slice(s * NT, (s + 1) * NT)
                        sv = svs[pi][:, m, sl]
                        sp = spp.tile([64, NT], BF16, tag="sp", name="sp")
                        nc.vector.tensor_mul(sp, sv[0:64, :],
                                             trig_t[:, 2 + pi, sl])
                        cp = spp.tile([64, NT], BF16, tag="cp", name="cp")
                        nc.vector.tensor_mul(cp, sv[0:64, :],
                                             trig_t[:, pi, sl])
                        ps_rot = psout.tile([64, NT], F32, tag="out",
                                            name="ps_rot")
                        nc.tensor.matmul(ps_rot, perm_t[:, 0, :], sp,
                                         start=True, stop=False)
                        nc.tensor.matmul(ps_rot, perm_t[:, 1, :], cp,
                                         start=False, stop=True)
                        rbc = rbcs[cb]
                        nc.vector.scalar_tensor_tensor(
                            fins[pi][0:64, m, sl], ps_rot, 1.0,
                            rbc[0:64, :],
                            mybir.AluOpType.mult, mybir.AluOpType.mult)
                        nc.vector.scalar_tensor_tensor(
                            fins[pi][64:128, m, sl], sv[64:128, :],
                            snw_t[64:128, pi:pi + 1], rbc[64:128, :],
                            mybir.AluOpType.mult, mybir.AluOpType.mult)
                    gatez = smp.tile([128, 1], F32, tag="gate", name="gatez")
                    nc.vector.tensor_scalar_mul(
                        gatez, rbcs[combos[-1]][:, 0:1], 0.0)
                    return gatez

                def v_transpose(b):
                    for m in range(MPC):
                        nc.sync.dma_start_transpose(
                            vtr[:, m, b],
                            vv[:, m, b * NT:(b + 1) * NT])

                def phaseC(t, interleave=None):
                    qsl = slice(t * NT, (t + 1) * NT)
                    nch = 4 * (t + 1)
                    attn_m = []
                    for m in range(MPC):
                        ps_attn = psacc.tile([128, NT], F32, tag="acc",
                                             name="ps_attn")
                        ps_sum = pssum.tile([1, NT], F32, tag="sum1",
                                            name="ps_sum")

                        def qk(kc):
                            ps_s = pssm.tile([128, NT], F32, tag="sm",
                                             name="ps_s")
                            nc.tensor.matmul(
                                ps_s,
                                kfT[:, m, kc * 128:(kc + 1) * 128],
                                qfT[:, m, qsl], start=True, stop=True)
                            e = expp.tile([128, NT], BF16, tag="e", name="e")
                            nc.scalar.activation(
                                e, ps_s, mybir.ActivationFunctionType.Exp,
                                scale=inv_sqrt_hd)
                            dd = kc * 128 - t * NT
                            if dd >= 0:  # diagonal chunk: causal mask
                                nc.vector.tensor_mul(
                                    e, e, mask4_t[:, 384 - dd:896 - dd])
                            return e

                        epipe = [qk(kc) for kc in range(min(3, nch))]
                        for kc in range(nch):
                            if kc + 3 < nch:
                                epipe.append(qk(kc + 3))
                            e = epipe.pop(0)
                            b, c = kc // 4, kc % 4
                            nc.tensor.matmul(
                                ps_attn, vtr[:, m, b, c, :], e,
                                start=(kc == 0), stop=(kc == nch - 1))
                            nc.tensor.matmul(
                                ps_sum, ones_hd, e,
                                start=(kc == 0), stop=(kc == nch - 1))
                        # normalize by 1/sumexp via stride-0 DMA broadcast
                        rrf = smp.tile([1, NT], F32, tag="rrf", name="rrf")
                        nc.vector.reciprocal_approx_fast(rrf, ps_sum)
                        rrc = smp.tile([1, NT], BF16, tag="rrc", name="rrc")
                        nc.vector.tensor_copy(rrc, rrf)
                        rbc = rbcp.tile([128, NT], BF16, tag="rbc",
                                        name="rbcC")
                        nc.gpsimd.partition_broadcast(rbc, rrc)
                        am = attnp.tile([128, NT], BF16, tag="am", name="am")
                        nc.vector.tensor_mul(am, ps_attn, rbc)
                        attn_m.append(am)
                        if interleave:
                            interleave.pop(0)()
                    # output projection (wo resident)
                    for i in range(KD):
                        ps_o = psout.tile([128, NT], F32, tag="out",
                                          name="ps_o")
                        for j in range(MPC):
                            nc.tensor.matmul(
                                ps_o, woT_t[:, j, i * 128:(i + 1) * 128],
                                attn_m[j], start=(j == 0),
                                stop=(j == MPC - 1))
                        ost = ostp.tile([128, NT], BF16, tag="ost",
                                        name="ost")
                        if i % 4 == 3:
                            nc.scalar.activation(
                                ost, ps_o, mybir.ActivationFunctionType.Copy)
                        else:
                            nc.vector.tensor_copy(ost, ps_o)
                        deng = nc.sync if i % 2 == 0 else nc.gpsimd
                        deng.dma_start(outT[i * 128:(i + 1) * 128, qsl],
                                       ost)
                        if interleave:
                            interleave.pop(0)()

                # ================= emission schedule =================
                sqtiles = {}
                xb0 = phaseA_loads(0, first=True)
                xb1 = phaseA_loads(1)
                phaseA_mms(0, xb0, "v")
                phaseA_mms(1, xb1, "v")
                xb2 = phaseA_loads(2)
                xb3 = phaseA_loads(3)
                phaseBh(0, sqtiles)       # conv/silu/sq for t in [0, 1024)
                phaseA_mms(2, xb2, "v")
                phaseBq(2, sqtiles)       # t in [1024, 1536) (needs A2)
                phaseA_mms(3, xb3, "v")
                gate = phaseBs_pair(0, sqtiles)
                phaseBq(3, sqtiles, gate)  # t in [1536, 2048)
                v_transpose(0)
                v_transpose(1)
                phaseBs_pair(2, sqtiles)
                v_transpose(2)
                v_transpose(3)
                phaseC(0)
                phaseC(1)
                phaseC(2)
                phaseC(3)
                if _DEBUG:
                    nc.sync.dma_start(dbg["dbg_qf"], qfT)
                    nc.sync.dma_start(dbg["dbg_kf"], kfT)
                    nc.sync.dma_start(dbg["dbg_vtr"], vtr)
                    nc.sync.dma_start(dbg["dbg_svq"], svq)
                    nc.sync.dma_start(dbg["dbg_rawq"], rawq)

    nc.compile()
    return nc


def _prep_inputs(hidden_states, cos, sin, Wq, Wk, Wv, Wo,
                 conv_q_w, conv_k_w, conv_v_w, q_norm_w, k_norm_w):
    f = np.float32
    bf = ml_dtypes.bfloat16
    x = np.asarray(hidden_states, f)[0]            # [T, D]
    xT = np.ascontiguousarray(x.T.astype(bf))      # [D, T] bf16
    WqT = np.ascontiguousarray(np.asarray(Wq, f).T.astype(bf))
    WkT = np.ascontiguousarray(np.asarray(Wk, f).T.astype(bf))
    WvT = np.ascontiguousarray(np.asarray(Wv, f).T.astype(bf))
    WoT = np.asarray(Wo, f).T                      # [CPC(full D), D]

    cosT = np.asarray(cos, f)[0].T                 # [RD, T]
    sinT = np.asarray(sin, f)[0].T
    nwq = np.asarray(q_norm_w, f)
    nwk = np.asarray(k_norm_w, f)

    # trig tables with norm weights folded into the rotary rows.
    # sin table indexed by SOURCE row r (out row p = r xor 32):
    #   r in 0:32  -> p = r+32: +sin[p]*nw[p]
    #   r in 32:64 -> p = r-32: -sin[p]*nw[p]
    def mk_trig(nw):
        cosb = cosT * nw[0:RD, None]
        ss = np.zeros((RD, T), f)
        ss[0:32] = sinT[32:64] * nw[32:64, None]
        ss[32:64] = -sinT[0:32] * nw[0:32, None]
        return cosb, ss

    cosq, ssq = mk_trig(nwq)
    cosk, ssk = mk_trig(nwk)
    trig = np.stack([cosq, cosk, ssq, ssk], axis=1).astype(bf)  # [64,4,T]

    snw = np.ones((128, 2), f)
    snw[RD:128, 0] = nwq[RD:128]
    snw[RD:128, 1] = nwk[RD:128]

    # causal mask strip: mask[kl, j] = 1.0 iff kl <= j - 384
    pp = np.arange(128, dtype=f)[:, None]
    jj = np.arange(896, dtype=f)[None, :]
    mask4 = (pp <= jj - 384).astype(bf)

    # rope rotate-half permutation + identity (lhsT: out = lhsT.T @ x)
    perm = np.zeros((64, 2, 64), f)
    for r in range(64):
        perm[r, 0, r ^ 32] = 1.0
        perm[r, 1, r] = 1.0
    perm = perm.astype(bf)

    in_maps = []
    for ci in range(NCORES):
        sl = slice(ci * CPC, (ci + 1) * CPC)
        convw = np.zeros((128, 3, MPC, KCONV), f)
        for pi, cw in enumerate((conv_q_w, conv_k_w, conv_v_w)):
            convw[:, pi] = np.asarray(cw, f)[sl].reshape(MPC, 128, KCONV
                                                         ).transpose(1, 0, 2)
        wo_res = np.ascontiguousarray(
            WoT[sl].reshape(MPC, 128, D).transpose(1, 0, 2).astype(bf))
        in_maps.append({
            "xT": xT,
            "wqT": np.ascontiguousarray(WqT[:, sl]),
            "wkT": np.ascontiguousarray(WkT[:, sl]),
            "wvT": np.ascontiguousarray(WvT[:, sl]),
            "woT": wo_res,
            "trig": trig,
            "snw": snw,
            "convw": np.ascontiguousarray(convw),
            "mask4": np.ascontiguousarray(mask4),
            "perm": perm,
        })
    return in_maps


def kernel(hidden_states, cos, sin, Wq, Wk, Wv, Wo,
           conv_q_w, conv_k_w, conv_v_w, q_norm_w, k_norm_w,
           _trace=False):
    global _COMPILED
    if _COMPILED is None:
        _COMPILED = _build()
    nc = _COMPILED
    in_maps = _prep_inputs(hidden_states, cos, sin, Wq, Wk, Wv, Wo,
                           conv_q_w, conv_k_w, conv_v_w, q_norm_w, k_norm_w)
    res = bass_utils.run_bass_kernel_spmd(
        nc, in_maps, core_ids=list(range(NCORES)), trace=_trace)
    if _DEBUG:
        global _DEBUG_RESULTS
        _DEBUG_RESULTS = res.results
    acc = np.zeros((D, T), np.float64)
    for r in res.results:
        acc += np.asarray(r["outT"], np.float64)
    out = np.ascontiguousarray(acc.T.astype(np.float32))[None]
    if _trace:
        kernel._last_results = res
    return out
